# revision 4
# baseline (speedup 1.0000x reference)
"""AdaFGL Bass kernel for 8 TRN2 NeuronCores (v3).

Row-shards N=6144 nodes across 8 cores (768 each). Design vs v2:

- re computed from t-hi fp16 ONLY (the lo-correction pass is dropped;
  measured end-to-end l2 5.1e-3 vs the 2e-2 gate).  Halves the re
  matmul work and the tT share of the AG payload.
- q stored as fp8(256*relu(re-m)): the x256 scale moves small q out of
  the fp8e4m3 subnormal range, so c = [re>=m] becomes recoverable as
  is_ge(q8,eps) FROM THE fp8 TILE.  The c pass therefore reads SBUF
  (not PSUM) and runs on the otherwise-idle GpSimd engine, one
  [128,768] op per (iter,h).  The 1/256 unscale is folded into the
  qe transpose identity (I/256), so the epilogue is unchanged.
- With c uniform {0,1} (no Sign halves) the whole Ue machinery
  (colsum(emb), the (se+Ue)/2 fold, the AG2 ue payload + broadcasts)
  is deleted; AG2 carries only mx.
- emb fp8 hi/lo planes are PACKED [hi|lo] per tile into one e8pack
  tile, used as a 256-wide stationary in the DoubleRow acc matmuls:
  qe (and ce) vs BOTH planes in ONE pass each -> 4 acc matmuls/iter
  instead of 8.  The hi+lo combine rides the existing epilogue
  transposes ([128,128] chunks, then one [128,64] add per tile).
- Relation-pass engine split: ACT 4x relu-q8, DVE 4x max-reduce,
  GpSimd 2x c-derive; PE 4 re + 4 acc matmuls per iter.
- A dummy 64B AllGather issues at t~0 so the runtime's first-collective
  barrier (39us!) runs in the shadow of the hete MLP instead of
  delaying AG1.
"""

import sys, os
sys.path.insert(0, "/opt/trn_rl_repo")

import numpy as np
from contextlib import ExitStack

from concourse import bass, bacc, tile, mybir
from concourse.bass_utils import run_bass_kernel_spmd

F32 = mybir.dt.float32
F16 = mybir.dt.float16
F8 = mybir.dt.float8e4
AX = mybir.AxisListType
OP = mybir.AluOpType
AF = mybir.ActivationFunctionType
PM = mybir.MatmulPerfMode

N = 6144
NCORES = 8
P = N // NCORES            # 768 rows per core
FEAT = 128
INSM = 512
HID = 256
OUT = 64
NT = P // 128              # 6 row tiles per core
NJ = N // 128              # 48 column chunks
INV_N2 = 1.0 / float(N * N)
BIG = 1.0e6
QS = 256.0                 # q8 scale (fp8e4m3 max 448 >> 256*max(q))

# merged AG payload layout (f32 words)
OFF_T = 0                  # t hi fp16 [64, 768] -> 24576 w
OFF_E = 24576              # e8pack fp8 [128, 768] -> 24576 w
OFF_W = OFF_E + 24576      # wr f32 [64, 64] -> 4096 w
OFF_U = OFF_W + 4096       # u f32 [1, 64]
AGW = OFF_U + 64

_CACHE = {}


def _build():
    nc = bacc.Bacc("TRN2", target_bir_lowering=False, debug=False,
                   num_devices=NCORES)

    def din(name, shape, dt=F32):
        return nc.dram_tensor(name, list(shape), dt, kind="ExternalInput").ap()

    def dout(name, shape):
        return nc.dram_tensor(name, list(shape), F32, kind="ExternalOutput").ap()

    xT_sm = din("xT_sm", (INSM, P), F16)
    xT_ori = din("xT_ori", (FEAT, P), F16)
    Wh0 = din("Wh0", (INSM, HID), F16); Wh1 = din("Wh1", (HID, OUT), F16)
    Ws0 = din("Ws0", (INSM, HID), F16); Ws1 = din("Ws1", (HID, OUT), F16)
    Wl0 = din("Wl0", (FEAT, HID), F16); Wl1 = din("Wl1", (HID, OUT), F16)
    bh0 = din("bh0", (HID,)); nbh0 = din("nbh0", (HID,))
    bs0 = din("bs0", (HID,)); nbs0 = din("nbs0", (HID,))
    bl0 = din("bl0", (HID,)); nbl0 = din("nbl0", (HID,))
    bh1b = din("bh1b", (128, OUT))
    bs1b = din("bs1b", (128, OUT))
    bl1b = din("bl1b", (128, OUT))
    naH_b = din("naH_b", (128, 1))
    naM_b = din("naM_b", (128, 1))
    aH_b = din("aH_b", (128, 1))
    ident16_d = din("ident16", (128, 128), F16)
    identS_d = din("identS", (128, 128), F16)   # I/256 for qe unscale
    ident32_d = din("ident32", (128, 128))
    ones_col_d = din("ones_col", (128, 1))
    colmask_d = din("colmask", (108,))

    out_ori = dout("out_ori", (P, OUT))
    out_smooth = dout("out_smooth", (P, OUT))
    out_msg = dout("out_msg", (P, OUT))

    agc_in = nc.dram_tensor("agc_in", [1, AGW], F32).ap()
    agc_out = nc.dram_tensor("agc_out", [NCORES, AGW], F32,
                             addr_space="Shared").ap()
    ag2_in = nc.dram_tensor("ag2_in", [1, 16], F32).ap()
    ag2_out = nc.dram_tensor("ag2_out", [NCORES, 16], F32,
                             addr_space="Shared").ap()
    agd_in = nc.dram_tensor("agd_in", [1, 16], F32).ap()
    agd_out = nc.dram_tensor("agd_out", [NCORES, 16], F32,
                             addr_space="Shared").ap()

    def esl(i):
        return slice(OUT * i, OUT * (i + 1))

    with tile.TileContext(nc) as tc, ExitStack() as ctx:
        # dummy collective first: absorbs the runtime's first-collective
        # barrier into the MLP window
        nc.gpsimd.collective_compute(
            "AllGather", OP.bypass,
            ins=[agd_in[:, :]],
            outs=[agd_out[:, :]],
            replica_groups=[list(range(NCORES))])

        cp = ctx.enter_context(tc.tile_pool(name="const", bufs=1))
        qcp = ctx.enter_context(tc.tile_pool(name="qc", bufs=3))

        # ---------- load constants / weights (hete path first) ----------
        def loadw(dram_ap, rows, cols, tag, eng):
            tiles = []
            for i in range(rows // 128):
                t = cp.tile([128, cols], F16, tag=f"{tag}_{i}",
                            name=f"{tag}_{i}")
                eng.dma_start(out=t[:, :],
                              in_=dram_ap[128 * i:128 * (i + 1), :])
                tiles.append(t)
            return tiles

        def loadb(dram_ap, ndram_ap, tag, eng):
            tiles = []
            for i in range(HID // 128):
                t = cp.tile([128, 1], F32, tag=f"{tag}_{i}", name=f"{tag}_{i}")
                eng.dma_start(out=t[:, :],
                              in_=dram_ap[128 * i:128 * (i + 1)])
                n = cp.tile([128, 1], F32, tag=f"{tag}n_{i}",
                            name=f"{tag}n_{i}")
                eng.dma_start(out=n[:, :],
                              in_=ndram_ap[128 * i:128 * (i + 1)])
                tiles.append((t, n))
            return tiles

        # x first on sync queue so the hete MLP can start ASAP
        XT_sm = []
        for f in range(INSM // 128):
            t = cp.tile([128, P], F16, tag=f"xts_{f}", name=f"xts_{f}")
            nc.sync.dma_start(out=t[:, :],
                              in_=xT_sm[128 * f:128 * (f + 1), :])
            XT_sm.append(t)
        Wh0_t = loadw(Wh0, INSM, HID, "Wh0", nc.scalar)
        Wh1_t = loadw(Wh1, HID, OUT, "Wh1", nc.scalar)
        bh0_t = loadb(bh0, nbh0, "bh0", nc.sync)
        bh1_b = cp.tile([128, OUT], F32, tag="bh1b", name="bh1b")
        nc.sync.dma_start(out=bh1_b[:, :], in_=bh1b[:, :])
        naH = cp.tile([128, 1], F32, tag="naH", name="naH")
        nc.sync.dma_start(out=naH[:, :], in_=naH_b[:, :])
        ident16 = cp.tile([128, 128], F16, tag="i16", name="i16")
        nc.sync.dma_start(out=ident16[:, :], in_=ident16_d[:, :])
        identS = cp.tile([128, 128], F16, tag="iS", name="iS")
        nc.sync.dma_start(out=identS[:, :], in_=identS_d[:, :])
        ones_col = cp.tile([128, 1], F32, tag="onec", name="onec")
        nc.sync.dma_start(out=ones_col[:, :], in_=ones_col_d[:, :])

        # fill-phase constants (gpsimd queue: idle until the AG trigger,
        # and these all land long before it)
        XT_ori = cp.tile([128, P], F16, tag="xto", name="xto")
        nc.gpsimd.dma_start(out=XT_ori[:, :], in_=xT_ori[:, :])
        Ws0_t = loadw(Ws0, INSM, HID, "Ws0", nc.gpsimd)
        Ws1_t = loadw(Ws1, HID, OUT, "Ws1", nc.gpsimd)
        Wl0_t = loadw(Wl0, FEAT, HID, "Wl0", nc.gpsimd)
        Wl1_t = loadw(Wl1, HID, OUT, "Wl1", nc.gpsimd)
        bs0_t = loadb(bs0, nbs0, "bs0", nc.gpsimd)
        bl0_t = loadb(bl0, nbl0, "bl0", nc.gpsimd)
        bs1_b = cp.tile([128, OUT], F32, tag="bs1b", name="bs1b")
        nc.gpsimd.dma_start(out=bs1_b[:, :], in_=bs1b[:, :])
        bl1_b = cp.tile([128, OUT], F32, tag="bl1b", name="bl1b")
        nc.gpsimd.dma_start(out=bl1_b[:, :], in_=bl1b[:, :])
        naM = cp.tile([128, 1], F32, tag="naM", name="naM")
        nc.gpsimd.dma_start(out=naM[:, :], in_=naM_b[:, :])
        aH = cp.tile([128, 1], F32, tag="aH", name="aH")
        nc.gpsimd.dma_start(out=aH[:, :], in_=aH_b[:, :])
        ident32 = cp.tile([128, 128], F32, tag="i32", name="i32")
        nc.gpsimd.dma_start(out=ident32[:, :], in_=ident32_d[:, :])
        colmask = cp.tile([108, 1], F32, tag="cmask", name="cmask")
        nc.gpsimd.dma_start(out=colmask[:, :], in_=colmask_d[:])

        # ---------- persistent tiles ----------
        emb_loc = cp.tile([128, NT * OUT], F32, tag="emb_loc", name="emb_loc")
        tT_stack = cp.tile([64, P], F16, tag="tT_stack", name="tT_stack")
        e8pack_loc = cp.tile([128, NT * 128], F8, tag="e8l", name="e8l")
        maxs = cp.tile([128, 108], F32, tag="maxs", name="maxs")
        u_sb = cp.tile([1, OUT], F32, tag="u_sb", name="u_sb")
        wr_sb = cp.tile([64, 64], F32, tag="wr_sb", name="wr_sb")

        # ---------- generic fp16 MLP ----------
        def mlp16(XT_tiles, W0_t, b0_t, W1_t, b1_b, na_b, out_wide, pfx,
                  tile_cb=None):
            nh = HID // 128
            with tc.tile_pool(name=pfx + "_h", bufs=1) as hp, \
                 tc.tile_pool(name=pfx + "_r", bufs=2) as rp, \
                 tc.tile_pool(name=pfx + "_ps1", bufs=2, space="PSUM") as ps1, \
                 tc.tile_pool(name=pfx + "_ps2", bufs=2, space="PSUM") as ps2:
                h_tiles = [hp.tile([128, P], F16, tag=f"h{hs}",
                                   name=f"{pfx}h{hs}") for hs in range(nh)]
                for ns in range(2):
                    for hs in range(nh):
                        sl = slice(384 * ns, 384 * (ns + 1))
                        pp = ps1.tile([128, 384], F32, tag="l1", name="l1")
                        nf = len(XT_tiles)
                        for fc in range(nf):
                            nc.tensor.matmul(
                                pp[:, :],
                                W0_t[fc][:, 128 * hs:128 * (hs + 1)],
                                XT_tiles[fc][:, sl],
                                start=(fc == 0), stop=(fc == nf - 1))
                        r1 = rp.tile([128, 384], F32, tag="r1", name="r1")
                        nc.scalar.activation(r1[:, :], pp[:, :], AF.Relu,
                                             bias=b0_t[hs][0][:, :], scale=1.0)
                        r2 = rp.tile([128, 384], F32, tag="r2", name="r2")
                        nc.scalar.activation(r2[:, :], pp[:, :], AF.Relu,
                                             bias=b0_t[hs][1][:, :], scale=-1.0)
                        nc.vector.scalar_tensor_tensor(
                            h_tiles[hs][:, sl], r2[:, :], na_b[:, :],
                            r1[:, :], op0=OP.mult, op1=OP.add)
                for i in range(NT):
                    pp = ps2.tile([128, OUT], F32, tag="l2", name="l2")
                    for hs in range(nh):
                        nc.tensor.matmul(
                            pp[:, :],
                            h_tiles[hs][:, 128 * i:128 * (i + 1)],
                            W1_t[hs][:, :],
                            start=(hs == 0), stop=(hs == nh - 1))
                    nc.vector.tensor_add(out_wide[:, esl(i)], pp[:, :],
                                         b1_b[:, :])
                    if tile_cb is not None:
                        tile_cb(i)

        # ---------- hete MLP with fused per-tile exp chain ----------
        # t = softmax(e)/||softmax(e)|| = exp(e-max)/||exp(e-max)|| -- the
        # softmax divide cancels, and the rsqrt is batched after the loop
        # (per-tile Sqrt would thrash the activation table, 1.3us/reload).
        with tc.tile_pool(name="smax", bufs=2) as sp, \
             tc.tile_pool(name="ps_wu", bufs=1, space="PSUM") as pswu, \
             tc.tile_pool(name="ps_ttr", bufs=2, space="PSUM") as pstr:
            ps_wr = pswu.tile([64, 64], F32, tag="pswr", name="pswr")
            ps_u = pswu.tile([1, OUT], F32, tag="psu", name="psu")
            v_w = cp.tile([128, NT * OUT], F32, tag="v_w", name="v_w")
            e16_w = cp.tile([128, NT * OUT], F16, tag="e16w", name="e16w")
            dsq = cp.tile([128, NT], F32, tag="dsq", name="dsq")
            vsq = cp.tile([128, OUT], F32, tag="vsq", name="vsq")

            def hete_tile_cb(i):
                e = emb_loc[:, esl(i)]
                rmx = sp.tile([128, 1], F32, tag="rmx", name="rmx")
                nc.vector.tensor_reduce(rmx[:, :], e, axis=AX.X, op=OP.max,
                                        negate=True)
                v = v_w[:, esl(i)]
                nc.scalar.activation(v, e, AF.Exp, bias=rmx[:, :], scale=1.0)
                nc.scalar.activation(vsq[:, :], v, AF.Square,
                                     accum_out=dsq[:, i:i + 1])
                nc.vector.tensor_copy(e16_w[:, esl(i)], e)
                # emb8 hi/lo packed [hi|lo] per tile for the acc matmuls
                hi8 = e8pack_loc[:, 128 * i:128 * i + 64]
                nc.vector.tensor_copy(hi8, e)
                nc.vector.tensor_sub(e8pack_loc[:, 128 * i + 64:128 * (i + 1)],
                                     e, hi8)

            mlp16(XT_sm, Wh0_t, bh0_t, Wh1_t, bh1_b, naH, emb_loc, "hete",
                  tile_cb=hete_tile_cb)
            # batched normalize + fp16 cast + transpose
            rdw = sp.tile([128, NT], F32, tag="rdw", name="rdw")
            nc.vector.reciprocal(rdw[:, :], dsq[:, :])
            isdw = sp.tile([128, NT], F32, tag="isdw", name="isdw")
            nc.scalar.activation(isdw[:, :], rdw[:, :], AF.Sqrt)
            for i in range(NT):
                t_i = sp.tile([128, OUT], F32, tag="t_i", name="t_i")
                nc.vector.tensor_scalar(t_i[:, :], v_w[:, esl(i)],
                                        isdw[:, i:i + 1], None, OP.mult)
                nc.tensor.matmul(ps_u[:, :], ones_col[:, :], t_i[:, :],
                                 start=(i == 0), stop=(i == NT - 1))
                hi = sp.tile([128, OUT], F16, tag="hi", name="hi")
                nc.vector.tensor_copy(hi[:, :], t_i[:, :])
                nc.tensor.matmul(ps_wr[:, :], hi[:, :], e16_w[:, esl(i)],
                                 start=(i == 0), stop=(i == NT - 1))
                csl = slice(128 * i, 128 * (i + 1))
                pt = pstr.tile([64, 128], F16, tag="ttr", name="ttr")
                nc.tensor.transpose(pt[:, :], hi[:, :], ident16[:, :])
                nc.scalar.copy(tT_stack[0:64, csl], pt[:, :])
            nc.scalar.copy(wr_sb[:, :], ps_wr[:, :])
            nc.scalar.copy(u_sb[:, :], ps_u[:, :])

        # ---------- pack + merged AllGather ----------
        nc.sync.dma_start(out=agc_in[0:1, OFF_T:OFF_T + 24576],
                          in_=tT_stack[:, :].bitcast(F32))
        nc.scalar.dma_start(out=agc_in[0:1, OFF_E:OFF_E + 24576],
                            in_=e8pack_loc[:, :].bitcast(F32))
        nc.sync.dma_start(out=agc_in[0:1, OFF_W:OFF_W + 4096],
                          in_=wr_sb[:, :])
        nc.scalar.dma_start(out=agc_in[0:1, OFF_U:OFF_U + OUT],
                            in_=u_sb[:, :])
        nc.gpsimd.collective_compute(
            "AllGather", OP.bypass,
            ins=[agc_in[:, :]],
            outs=[agc_out[:, :]],
            replica_groups=[list(range(NCORES))])

        # ---------- fill the AG window ----------
        # own-block max tiles (diag suppressed), raw maxes
        with tc.tile_pool(name="ps_rex", bufs=2, space="PSUM") as psre0:
            for s in range(NT):
                for h in range(2):
                    sl = slice(384 * h, 384 * (h + 1))
                    pp = psre0.tile([128, 384], F32, tag="re", name="rex")
                    nc.tensor.matmul(pp[:, :],
                                     tT_stack[:, 128 * s:128 * (s + 1)],
                                     tT_stack[:, sl], start=True, stop=True)
                    if (s // 3) == h:
                        off = 128 * s - 384 * h
                        nc.vector.scalar_tensor_tensor(
                            pp[:, off:off + 128], ident32[:, :], -BIG,
                            pp[:, off:off + 128], op0=OP.mult, op1=OP.add)
                    slot = 96 + 2 * s + h
                    nc.vector.tensor_reduce(maxs[:, slot:slot + 1], pp[:, :],
                                            axis=AX.X, op=OP.max)

        # smooth + ori MLPs
        with tc.tile_pool(name="mlpout", bufs=1) as mo:
            sm_out = mo.tile([128, NT * OUT], F32, tag="smo", name="smo")

            def sm_cb(i):
                nc.scalar.dma_start(out=out_smooth[128 * i:128 * (i + 1), :],
                                    in_=sm_out[:, esl(i)])

            mlp16(XT_sm, Ws0_t, bs0_t, Ws1_t, bs1_b, naM, sm_out, "smooth",
                  tile_cb=sm_cb)

            or_out = mo.tile([128, NT * OUT], F32, tag="oro", name="oro")

            def or_cb(i):
                nc.scalar.dma_start(out=out_ori[128 * i:128 * (i + 1), :],
                                    in_=or_out[:, esl(i)])

            mlp16([XT_ori], Wl0_t, bl0_t, Wl1_t, bl1_b, naM, or_out, "ori",
                  tile_cb=or_cb)

        # ---------- unpack ----------
        tf_blk = [cp.tile([64, P], F16, tag=f"tf_{k}", name=f"tf_{k}")
                  for k in range(NCORES)]
        e8pack = [cp.tile([128, NT * 128], F8, tag=f"e8_{k}", name=f"e8_{k}")
                  for k in range(NCORES)]
        mb = cp.tile([128, 1], F32, tag="mb", name="mb")
        nmb = cp.tile([128, 1], F32, tag="nmb", name="nmb")
        nm256 = cp.tile([128, 1], F32, tag="nm256", name="nm256")
        w_sb = cp.tile([64, 64], F32, tag="w_sb", name="w_sb")
        U_sb = cp.tile([1, OUT], F32, tag="U_sb", name="U_sb")
        m01 = cp.tile([1, 1], F32, tag="m01", name="m01")

        with tc.tile_pool(name="unpack", bufs=1) as up:
            # m chain first (tiny, unlocks q8 bias)
            usum = up.tile([1, NCORES * OUT], F32, tag="usum", name="usum")
            nc.sync.dma_start(out=usum[:, :],
                              in_=agc_out[0:NCORES, OFF_U:OFF_U + OUT])
            nc.vector.tensor_reduce(
                U_sb[:, :],
                usum[:, :].rearrange("a (k o) -> a o k", k=NCORES),
                axis=AX.X, op=OP.add)
            usq = up.tile([1, OUT], F32, tag="usq", name="usq")
            uu = up.tile([1, 1], F32, tag="uu", name="uu")
            nc.scalar.activation(usq[:, :], U_sb[:, :], AF.Square,
                                 accum_out=uu[:, :])
            nc.vector.tensor_scalar(m01[:, :], uu[:, :], -float(N), INV_N2,
                                    OP.add, OP.mult)
            nc.gpsimd.partition_broadcast(mb[:, :], m01[:, :])
            nc.vector.tensor_scalar(nmb[:, :], mb[:, :], -1.0, None, OP.mult)
            nc.vector.tensor_scalar(nm256[:, :], mb[:, :], -QS, None, OP.mult)
            # bulk unpack, all on the sync queue in consumption order
            for k in range(NCORES):
                nc.sync.dma_start(out=tf_blk[k][:, :].bitcast(F32),
                                  in_=agc_out[k:k + 1, OFF_T:OFF_T + 24576])
                nc.sync.dma_start(out=e8pack[k][:, :].bitcast(F32),
                                  in_=agc_out[k:k + 1, OFF_E:OFF_E + 24576])
            # w sum (needed only post-relation, for tw)
            wsum = up.tile([64, NCORES * 64], F32, tag="wsum", name="wsum")
            for k in range(NCORES):
                nc.sync.dma_start(
                    out=wsum[:, 64 * k:64 * (k + 1)],
                    in_=agc_out[k:k + 1, OFF_W:OFF_W + 4096])
            nc.vector.tensor_reduce(
                w_sb[:, :],
                wsum[:, :].rearrange("p (k o) -> p o k", k=NCORES),
                axis=AX.X, op=OP.add)

        # ---------- fused relation + propagation pass ----------
        qeT16 = cp.tile([128, P], F16, tag="qeT", name="qeT")
        ceT16 = cp.tile([128, P], F16, tag="ceT", name="ceT")
        with tc.tile_pool(name="ps_acc", bufs=1, space="PSUM") as pacc:
            qe_ps = [pacc.tile([128, 384], F32, tag=f"qe{h}", name=f"qe{h}")
                     for h in range(2)]
            ce_ps = [pacc.tile([128, 384], F32, tag=f"ce{h}", name=f"ce{h}")
                     for h in range(2)]
            with tc.tile_pool(name="ps_re", bufs=4, space="PSUM") as psre:
                for k in range(NCORES):
                    for pr in range(NT // 2):
                        q8p = [qcp.tile([128, 768], F8, tag=f"q8p{h}",
                                        name=f"q8p{h}") for h in range(2)]
                        c8p = [qcp.tile([128, 768], F8, tag=f"c8p{h}",
                                        name=f"c8p{h}") for h in range(2)]
                        for d in range(2):
                            sub = 2 * pr + d
                            j = NT * k + sub
                            stk = tf_blk[k][:, 128 * sub:128 * (sub + 1)]
                            for h in range(2):
                                sl = slice(384 * h, 384 * (h + 1))
                                dsl = slice(384 * d, 384 * (d + 1))
                                pp = psre.tile([128, 384], F32, tag="re",
                                               name="rem")
                                nc.tensor.matmul(pp[:, :], stk,
                                                 tT_stack[:, sl],
                                                 start=True, stop=True)
                                nc.scalar.activation(q8p[h][:, dsl], pp[:, :],
                                                     AF.Relu,
                                                     bias=nm256[:, :],
                                                     scale=QS)
                                slot = 2 * j + h
                                nc.vector.tensor_reduce(
                                    maxs[:, slot:slot + 1], pp[:, :],
                                    axis=AX.X, op=OP.max)
                        # c = [q8 > 0] from the scaled fp8 tiles (GpSimd)
                        for h in range(2):
                            nc.gpsimd.tensor_scalar(
                                c8p[h][:, :], q8p[h][:, :], 1e-4, None,
                                OP.is_ge)
                        first = (k == 0 and pr == 0)
                        last = (k == NCORES - 1 and pr == NT // 2 - 1)
                        lhs8 = e8pack[k][:, 256 * pr:256 * (pr + 1)].rearrange(
                            "a (two m) -> a two m", two=2)
                        for h in range(2):
                            rq = q8p[h][:, :].rearrange("a (two n) -> a two n",
                                                        two=2)
                            rc = c8p[h][:, :].rearrange("a (two n) -> a two n",
                                                        two=2)
                            nc.tensor.matmul(
                                qe_ps[h][:, :], lhs8, rq,
                                start=first, stop=last,
                                perf_mode=PM.DoubleRow,
                                skip_group_check=True)
                            nc.tensor.matmul(
                                ce_ps[h][:, :], lhs8, rc,
                                start=first, stop=last,
                                perf_mode=PM.DoubleRow,
                                skip_group_check=True)
            # qe/ce out of PSUM (fp16) so the psre pool can close
            for h in range(2):
                sl = slice(384 * h, 384 * (h + 1))
                nc.scalar.copy(qeT16[:, sl], qe_ps[h][:, :])
                nc.scalar.copy(ceT16[:, sl], ce_ps[h][:, :])

        # ---------- max stat -> AG2 ----------
        with tc.tile_pool(name="stats", bufs=1) as stp, \
             tc.tile_pool(name="ps_st", bufs=1, space="PSUM") as psst:
            ptm = psst.tile([108, 128], F32, tag="mtr", name="mtr")
            nc.tensor.transpose(ptm[:, :], maxs[:, :], ident32[:, :])
            mm_ = stp.tile([108, 128], F32, tag="mm", name="mm")
            nc.vector.tensor_scalar(mm_[:, :], ptm[:, :], colmask[:, :],
                                    None, OP.mult)
            mv = stp.tile([108, 1], F32, tag="mv", name="mv")
            nc.vector.tensor_reduce(mv[:, :], mm_[:, :], axis=AX.X,
                                    op=OP.max)
            ptm2 = psst.tile([1, 108], F32, tag="mtr2", name="mtr2")
            nc.tensor.transpose(ptm2[:, :], mv[:, :],
                                ident32[0:108, 0:108])
            mrow = stp.tile([1, 108], F32, tag="mrow", name="mrow")
            nc.scalar.copy(mrow[:, :], ptm2[:, :])
            mx01 = stp.tile([1, 1], F32, tag="mx01", name="mx01")
            nc.vector.tensor_reduce(mx01[:, :], mrow[:, :], axis=AX.X,
                                    op=OP.max)
            nc.sync.dma_start(out=ag2_in[0:1, 0:1], in_=mx01[:, :])
        nc.gpsimd.collective_compute(
            "AllGather", OP.bypass,
            ins=[ag2_in[:, :]],
            outs=[ag2_out[:, :]],
            replica_groups=[list(range(NCORES))])

        # ---------- AG2-window fill: tw, transposes, pre-ip epilogue ----
        if True:
            with tc.tile_pool(name="epi", bufs=1) as ep, \
                 tc.tile_pool(name="ps_epi", bufs=2, space="PSUM") as pse:
                # scalars derived from m
                imb = cp.tile([128, 1], F32, tag="imb", name="imb")
                nimb = cp.tile([128, 1], F32, tag="nimb", name="nimb")
                n1m = cp.tile([128, 1], F32, tag="n1m", name="n1m")
                nc.vector.reciprocal(imb[:, :], mb[:, :])
                nc.vector.tensor_scalar(nimb[:, :], imb[:, :], -1.0, None,
                                        OP.mult)
                nc.vector.tensor_scalar(n1m[:, :], mb[:, :], 1.0, -1.0,
                                        OP.mult, OP.add)
                # tw = t @ w  (fp16 t-hi)
                w16 = ep.tile([64, 64], F16, tag="w16", name="w16")
                nc.vector.tensor_copy(w16[:, :], w_sb[:, :])
                tw_nm = ep.tile([128, NT * OUT], F32, tag="tw", name="tw")
                for i in range(NT):
                    ptw = pse.tile([128, OUT], F32, tag="ptw", name="ptw")
                    nc.tensor.matmul(ptw[:, :],
                                     tT_stack[:, 128 * i:128 * (i + 1)],
                                     w16[:, :], start=True, stop=True)
                    nc.scalar.copy(tw_nm[:, esl(i)], ptw[:, :])
                # qe/ce to node-major via fp16 transposes; hi+lo plane add.
                # identS = I/256 folds the q8 unscale into the qe transpose.
                qe_nm = ep.tile([128, NT * OUT], F32, tag="qe_nm",
                                name="qe_nm")
                ce_nm = ep.tile([128, NT * OUT], F32, tag="ce_nm",
                                name="ce_nm")
                for i in range(NT):
                    csl = slice(128 * i, 128 * (i + 1))
                    pq = pse.tile([128, 128], F16, tag="tq", name="tq")
                    nc.tensor.transpose(pq[:, :], qeT16[:, csl],
                                        identS[:, :])
                    qtmp = ep.tile([128, 128], F16, tag="qtmp", name="qtmp")
                    nc.scalar.copy(qtmp[:, :], pq[:, :])
                    nc.vector.tensor_add(qe_nm[:, esl(i)], qtmp[:, 0:64],
                                         qtmp[:, 64:128])
                    pc = pse.tile([128, 128], F16, tag="tc", name="tc")
                    nc.tensor.transpose(pc[:, :], ceT16[:, csl],
                                        ident16[:, :])
                    ctmp = ep.tile([128, 128], F16, tag="ctmp", name="ctmp")
                    nc.scalar.copy(ctmp[:, :], pc[:, :])
                    nc.vector.tensor_add(ce_nm[:, esl(i)], ctmp[:, 0:64],
                                         ctmp[:, 64:128])
                # G = (qe - tw)/m + ce ;  z1 = qe + (m-1)*emb
                G = ep.tile([128, NT * OUT], F32, tag="G", name="G")
                nc.vector.scalar_tensor_tensor(G[:, :], qe_nm[:, :],
                                               imb[:, :], ce_nm[:, :],
                                               op0=OP.mult, op1=OP.add)
                nc.vector.scalar_tensor_tensor(G[:, :], tw_nm[:, :],
                                               nimb[:, :], G[:, :],
                                               op0=OP.mult, op1=OP.add)
                z1 = ep.tile([128, NT * OUT], F32, tag="z1", name="z1")
                nc.vector.scalar_tensor_tensor(z1[:, :], emb_loc[:, :],
                                               n1m[:, :], qe_nm[:, :],
                                               op0=OP.mult, op1=OP.add)
                cpos = ep.tile([128, NT * OUT], F32, tag="cpos", name="cpos")
                nc.vector.scalar_tensor_tensor(cpos[:, :], G[:, :], aH[:, :],
                                               emb_loc[:, :], op0=OP.mult,
                                               op1=OP.add)
                cneg = ep.tile([128, NT * OUT], F32, tag="cneg", name="cneg")
                nc.vector.scalar_tensor_tensor(cneg[:, :], emb_loc[:, :],
                                               naH[:, :], G[:, :],
                                               op0=OP.mult, op1=OP.subtract)
                emb_half = ep.tile([128, NT * OUT], F32, tag="ehalf",
                                   name="ehalf")
                nc.vector.tensor_scalar(emb_half[:, :], emb_loc[:, :], 0.5,
                                        None, OP.mult)
                # pre-warm the Exp activation table during the AG2 window
                dmx = ep.tile([1, 1], F32, tag="dmx", name="dmx")
                nc.scalar.activation(dmx[:, :], m01[:, :], AF.Exp)

                # ---------- AG2 -> ip ----------
                ipb = cp.tile([128, 1], F32, tag="ipb", name="ipb")
                naip = cp.tile([128, 1], F32, tag="naip", name="naip")
                with tc.tile_pool(name="glob", bufs=1) as gp:
                    m8 = gp.tile([1, NCORES], F32, tag="m8", name="m8")
                    nc.sync.dma_start(out=m8[:, :], in_=ag2_out[:, 0:1])
                    mxg = gp.tile([1, 1], F32, tag="mxg", name="mxg")
                    nc.vector.tensor_reduce(mxg[:, :], m8[:, :], axis=AX.X,
                                            op=OP.max)
                    pd = gp.tile([1, 1], F32, tag="pd", name="pd")
                    nc.vector.tensor_sub(pd[:, :], mxg[:, :], m01[:, :])
                    ip01 = gp.tile([1, 1], F32, tag="ip01", name="ip01")
                    nc.vector.reciprocal(ip01[:, :], pd[:, :])
                    nc.gpsimd.partition_broadcast(ipb[:, :], ip01[:, :])
                    nc.vector.tensor_mul(naip[:, :], ipb[:, :], naH[:, :])

                pos_w = ep.tile([128, NT * OUT], F32, tag="pos_w",
                                name="pos_w")
                nc.vector.scalar_tensor_tensor(pos_w[:, :], z1[:, :],
                                               ipb[:, :], cpos[:, :],
                                               op0=OP.mult, op1=OP.add)
                neg_w = ep.tile([128, NT * OUT], F32, tag="neg_w",
                                name="neg_w")
                nc.vector.scalar_tensor_tensor(neg_w[:, :], z1[:, :],
                                               naip[:, :], cneg[:, :],
                                               op0=OP.mult, op1=OP.add)

                pp_w = ep.tile([128, NT * OUT], F32, tag="pp_w", name="pp_w")
                pn_w = ep.tile([128, NT * OUT], F32, tag="pn_w", name="pn_w")
                for src_w, dst in ((pos_w, pp_w), (neg_w, pn_w)):
                    rmx2 = ep.tile([128, NT], F32, tag="rmx2", name="rmx2")
                    nc.vector.tensor_reduce(
                        rmx2[:, :],
                        src_w[:, :].rearrange("p (g o) -> p g o", o=OUT),
                        axis=AX.X, op=OP.max, negate=True)
                    ex2 = ep.tile([128, NT * OUT], F32, tag="ex2", name="ex2")
                    ssum2 = ep.tile([128, NT], F32, tag="ssum2", name="ssum2")
                    for i in range(NT):
                        nc.scalar.activation(ex2[:, esl(i)], src_w[:, esl(i)],
                                             AF.Exp, bias=rmx2[:, i:i + 1],
                                             scale=1.0)
                    nc.vector.tensor_reduce(
                        ssum2[:, :],
                        ex2[:, :].rearrange("p (g o) -> p g o", o=OUT),
                        axis=AX.X, op=OP.add)
                    rs2 = ep.tile([128, NT], F32, tag="rs2", name="rs2")
                    nc.vector.reciprocal(rs2[:, :], ssum2[:, :])
                    for i in range(NT):
                        nc.vector.tensor_scalar(dst[:, esl(i)], ex2[:, esl(i)],
                                                rs2[:, i:i + 1], None, OP.mult)
                dd = ep.tile([128, NT * OUT], F32, tag="dd", name="dd")
                nc.vector.scalar_tensor_tensor(dd[:, :], pp_w[:, :], 0.5,
                                               emb_half[:, :], op0=OP.mult,
                                               op1=OP.add)
                msg = ep.tile([128, NT * OUT], F32, tag="msg", name="msg")
                nc.vector.scalar_tensor_tensor(msg[:, :], pn_w[:, :], -0.5,
                                               dd[:, :], op0=OP.mult,
                                               op1=OP.add)
                for i in range(NT):
                    eng = (nc.gpsimd, nc.sync, nc.scalar)[i % 3]
                    eng.dma_start(out=out_msg[128 * i:128 * (i + 1), :],
                                  in_=msg[:, esl(i)])

    nc.compile()
    return nc


def _get_nc():
    if "nc" not in _CACHE:
        _CACHE["nc"] = _build()
    return _CACHE["nc"]


def _make_in_maps(inputs):
    f = np.float32
    h = np.float16
    sm = np.ascontiguousarray(inputs["smoothed_feature"], dtype=f)
    ori = np.ascontiguousarray(inputs["ori_feature"], dtype=f)
    aM = float(np.asarray(inputs["prelu_model"]).reshape(-1)[0])
    aHv = float(np.asarray(inputs["prelu_hete"]).reshape(-1)[0])

    def b0pair(name):
        b = np.ascontiguousarray(inputs[name], dtype=f)
        return b, np.ascontiguousarray(-b, dtype=f)

    bh0v, nbh0v = b0pair("b_hete0")
    bs0v, nbs0v = b0pair("b_smooth0")
    bl0v, nbl0v = b0pair("b_local0")

    def b1b(name):
        return np.ascontiguousarray(
            np.broadcast_to(np.asarray(inputs[name], dtype=f), (128, OUT)))

    shared = {
        "Wh0": inputs["W_hete0"].astype(h), "Wh1": inputs["W_hete1"].astype(h),
        "Ws0": inputs["W_smooth0"].astype(h),
        "Ws1": inputs["W_smooth1"].astype(h),
        "Wl0": inputs["W_local0"].astype(h),
        "Wl1": inputs["W_local1"].astype(h),
        "bh0": bh0v, "nbh0": nbh0v, "bs0": bs0v, "nbs0": nbs0v,
        "bl0": bl0v, "nbl0": nbl0v,
        "bh1b": b1b("b_hete1"), "bs1b": b1b("b_smooth1"),
        "bl1b": b1b("b_local1"),
        "naH_b": np.full((128, 1), -aHv, dtype=f),
        "naM_b": np.full((128, 1), -aM, dtype=f),
        "aH_b": np.full((128, 1), aHv, dtype=f),
        "ident16": np.eye(128, dtype=h),
        "identS": (np.eye(128) / QS).astype(h),
        "ident32": np.eye(128, dtype=f),
        "ones_col": np.ones((128, 1), dtype=f),
    }
    shared = {k: np.ascontiguousarray(v) for k, v in shared.items()}
    in_maps = []
    for r in range(NCORES):
        cm = np.ones(108, dtype=f)
        cm[2 * NT * r:2 * NT * (r + 1)] = 0.0  # drop raw own-block slots
        m = dict(shared)
        m["xT_sm"] = np.ascontiguousarray(
            sm[P * r:P * (r + 1)].T.astype(h))
        m["xT_ori"] = np.ascontiguousarray(
            ori[P * r:P * (r + 1)].T.astype(h))
        m["colmask"] = cm
        in_maps.append(m)
    return in_maps


def _ensure_ntff_hook():
    """The agent image's antenv lacks axon_hooks; shim it so
    run_bass_kernel_spmd(trace=True) can capture NTFF profiles."""
    if "antenv.axon_hooks" in sys.modules:
        return
    import types
    import antenv
    mod = types.ModuleType("antenv.axon_hooks")
    state = {"hook": None}
    mod.set_axon_ntff_profile_hook = lambda h: state.__setitem__("hook", h)
    mod.get_axon_ntff_profile_hook = lambda: state["hook"]
    sys.modules["antenv.axon_hooks"] = mod
    antenv.axon_hooks = mod
    try:
        from trn_agent_boot.trn_boot import _ntff_profile_via_ctypes
        mod.set_axon_ntff_profile_hook(
            _ntff_profile_via_ctypes("/opt/axon/libaxon_pjrt.so"))
    except Exception as e:
        print(f"ntff hook install failed: {e}", file=sys.stderr)


def run(inputs, trace=False):
    if trace:
        _ensure_ntff_hook()
    nc = _get_nc()
    in_maps = _make_in_maps(inputs)
    res = run_bass_kernel_spmd(nc, in_maps, list(range(NCORES)), trace=trace)
    outs = res.results
    o1 = np.concatenate([outs[r]["out_ori"] for r in range(NCORES)], axis=0)
    o2 = np.concatenate([outs[r]["out_smooth"] for r in range(NCORES)], axis=0)
    o3 = np.concatenate([outs[r]["out_msg"] for r in range(NCORES)], axis=0)
    return (o1.astype(np.float32), o2.astype(np.float32),
            o3.astype(np.float32)), res


def kernel(**inputs):
    (o1, o2, o3), _ = run(inputs, trace=False)
    return (o1, o2, o3)


# revision 5
# speedup vs baseline: 3.2617x; 3.2617x over previous
"""AdaFGL Bass kernel for 8 TRN2 NeuronCores (v3).

Row-shards N=6144 nodes across 8 cores (768 each). Design vs v2:

- re computed from t-hi fp16 ONLY (the lo-correction pass is dropped;
  measured end-to-end l2 5.1e-3 vs the 2e-2 gate).  Halves the re
  matmul work and the tT share of the AG payload.
- q stored as fp8(256*relu(re-m)): the x256 scale moves small q out of
  the fp8e4m3 subnormal range, so c = [re>=m] becomes recoverable as
  is_ge(q8,eps) FROM THE fp8 TILE.  The c pass therefore reads SBUF
  (not PSUM) and runs on the otherwise-idle GpSimd engine, one
  [128,768] op per (iter,h).  The 1/256 unscale is folded into the
  qe transpose identity (I/256), so the epilogue is unchanged.
- With c uniform {0,1} (no Sign halves) the whole Ue machinery
  (colsum(emb), the (se+Ue)/2 fold, the AG2 ue payload + broadcasts)
  is deleted; AG2 carries only mx.
- emb fp8 hi/lo planes are PACKED [hi|lo] per tile into one e8pack
  tile, used as a 256-wide stationary in the DoubleRow acc matmuls:
  qe (and ce) vs BOTH planes in ONE pass each -> 4 acc matmuls/iter
  instead of 8.  The hi+lo combine rides the existing epilogue
  transposes ([128,128] chunks, then one [128,64] add per tile).
- Relation-pass engine split: ACT 4x relu-q8, DVE 4x max-reduce,
  GpSimd 2x c-derive; PE 4 re + 4 acc matmuls per iter.
- A dummy 64B AllGather issues at t~0 so the runtime's first-collective
  barrier (39us!) runs in the shadow of the hete MLP instead of
  delaying AG1.
"""

import sys, os
sys.path.insert(0, "/opt/trn_rl_repo")

import numpy as np
from contextlib import ExitStack

from concourse import bass, bacc, tile, mybir
from concourse.bass_utils import run_bass_kernel_spmd

F32 = mybir.dt.float32
F16 = mybir.dt.float16
F8 = mybir.dt.float8e4
AX = mybir.AxisListType
OP = mybir.AluOpType
AF = mybir.ActivationFunctionType
PM = mybir.MatmulPerfMode

N = 6144
NCORES = 8
P = N // NCORES            # 768 rows per core
FEAT = 128
INSM = 512
HID = 256
OUT = 64
NT = P // 128              # 6 row tiles per core
NJ = N // 128              # 48 column chunks
INV_N2 = 1.0 / float(N * N)
BIG = 1.0e6
QS = 256.0                 # q8 scale (fp8e4m3 max 448 >> 256*max(q))

# merged AG payload layout (f32 words)
OFF_T = 0                  # t hi fp16 [64, 768] -> 24576 w
OFF_E = 24576              # e8pack fp8 [128, 768] -> 24576 w
OFF_W = OFF_E + 24576      # wr f32 [64, 64] -> 4096 w
OFF_U = OFF_W + 4096       # u f32 [1, 64]
AGW = OFF_U + 64

_CACHE = {}


def _build():
    nc = bacc.Bacc("TRN2", target_bir_lowering=False, debug=False,
                   num_devices=NCORES)

    def din(name, shape, dt=F32):
        return nc.dram_tensor(name, list(shape), dt, kind="ExternalInput").ap()

    def dout(name, shape):
        return nc.dram_tensor(name, list(shape), F32, kind="ExternalOutput").ap()

    xT_sm = din("xT_sm", (INSM, P), F16)
    xT_ori = din("xT_ori", (FEAT, P), F16)
    Wh0 = din("Wh0", (INSM, HID), F16); Wh1 = din("Wh1", (HID, OUT), F16)
    Ws0 = din("Ws0", (INSM, HID), F16); Ws1 = din("Ws1", (HID, OUT), F16)
    Wl0 = din("Wl0", (FEAT, HID), F16); Wl1 = din("Wl1", (HID, OUT), F16)
    bh0 = din("bh0", (HID,)); nbh0 = din("nbh0", (HID,))
    bs0 = din("bs0", (HID,)); nbs0 = din("nbs0", (HID,))
    bl0 = din("bl0", (HID,)); nbl0 = din("nbl0", (HID,))
    bh1b = din("bh1b", (128, OUT))
    bs1b = din("bs1b", (128, OUT))
    bl1b = din("bl1b", (128, OUT))
    naH_b = din("naH_b", (128, 1))
    naM_b = din("naM_b", (128, 1))
    aH_b = din("aH_b", (128, 1))
    ident16_d = din("ident16", (128, 128), F16)
    identS_d = din("identS", (128, 128), F16)   # I/256 for qe unscale
    ident32_d = din("ident32", (128, 128))
    ones_col_d = din("ones_col", (128, 1))
    colmask_d = din("colmask", (108,))

    out_ori = dout("out_ori", (P, OUT))
    out_smooth = dout("out_smooth", (P, OUT))
    out_msg = dout("out_msg", (P, OUT))

    agc_in = nc.dram_tensor("agc_in", [1, AGW], F32).ap()
    agc_out = nc.dram_tensor("agc_out", [NCORES, AGW], F32,
                             addr_space="Shared").ap()
    ag2_in = nc.dram_tensor("ag2_in", [1, 16], F32).ap()
    ag2_out = nc.dram_tensor("ag2_out", [NCORES, 16], F32,
                             addr_space="Shared").ap()
    agd_in = nc.dram_tensor("agd_in", [1, 16], F32).ap()
    agd_out = nc.dram_tensor("agd_out", [NCORES, 16], F32,
                             addr_space="Shared").ap()

    def esl(i):
        return slice(OUT * i, OUT * (i + 1))

    with tile.TileContext(nc) as tc, ExitStack() as ctx:
        # dummy collective first: absorbs the runtime's first-collective
        # barrier into the MLP window
        nc.gpsimd.collective_compute(
            "AllGather", OP.bypass,
            ins=[agd_in[:, :]],
            outs=[agd_out[:, :]],
            replica_groups=[list(range(NCORES))])

        cp = ctx.enter_context(tc.tile_pool(name="const", bufs=1))
        qcp = ctx.enter_context(tc.tile_pool(name="qc", bufs=3))

        # ---------- load constants / weights (hete path first) ----------
        def loadw(dram_ap, rows, cols, tag, eng):
            tiles = []
            for i in range(rows // 128):
                t = cp.tile([128, cols], F16, tag=f"{tag}_{i}",
                            name=f"{tag}_{i}")
                eng.dma_start(out=t[:, :],
                              in_=dram_ap[128 * i:128 * (i + 1), :])
                tiles.append(t)
            return tiles

        def loadb(dram_ap, ndram_ap, tag, eng):
            tiles = []
            for i in range(HID // 128):
                t = cp.tile([128, 1], F32, tag=f"{tag}_{i}", name=f"{tag}_{i}")
                eng.dma_start(out=t[:, :],
                              in_=dram_ap[128 * i:128 * (i + 1)])
                n = cp.tile([128, 1], F32, tag=f"{tag}n_{i}",
                            name=f"{tag}n_{i}")
                eng.dma_start(out=n[:, :],
                              in_=ndram_ap[128 * i:128 * (i + 1)])
                tiles.append((t, n))
            return tiles

        # x first on sync queue so the hete MLP can start ASAP
        XT_sm = []
        for f in range(INSM // 128):
            t = cp.tile([128, P], F16, tag=f"xts_{f}", name=f"xts_{f}")
            nc.sync.dma_start(out=t[:, :],
                              in_=xT_sm[128 * f:128 * (f + 1), :])
            XT_sm.append(t)
        Wh0_t = loadw(Wh0, INSM, HID, "Wh0", nc.scalar)
        Wh1_t = loadw(Wh1, HID, OUT, "Wh1", nc.scalar)
        bh0_t = loadb(bh0, nbh0, "bh0", nc.sync)
        bh1_b = cp.tile([128, OUT], F32, tag="bh1b", name="bh1b")
        nc.sync.dma_start(out=bh1_b[:, :], in_=bh1b[:, :])
        naH = cp.tile([128, 1], F32, tag="naH", name="naH")
        nc.sync.dma_start(out=naH[:, :], in_=naH_b[:, :])
        ident16 = cp.tile([128, 128], F16, tag="i16", name="i16")
        nc.sync.dma_start(out=ident16[:, :], in_=ident16_d[:, :])
        identS = cp.tile([128, 128], F16, tag="iS", name="iS")
        nc.sync.dma_start(out=identS[:, :], in_=identS_d[:, :])
        ones_col = cp.tile([128, 1], F32, tag="onec", name="onec")
        nc.sync.dma_start(out=ones_col[:, :], in_=ones_col_d[:, :])

        # fill-phase constants (gpsimd queue: idle until the AG trigger,
        # and these all land long before it)
        XT_ori = cp.tile([128, P], F16, tag="xto", name="xto")
        nc.gpsimd.dma_start(out=XT_ori[:, :], in_=xT_ori[:, :])
        Ws0_t = loadw(Ws0, INSM, HID, "Ws0", nc.gpsimd)
        Ws1_t = loadw(Ws1, HID, OUT, "Ws1", nc.gpsimd)
        Wl0_t = loadw(Wl0, FEAT, HID, "Wl0", nc.gpsimd)
        Wl1_t = loadw(Wl1, HID, OUT, "Wl1", nc.gpsimd)
        bs0_t = loadb(bs0, nbs0, "bs0", nc.gpsimd)
        bl0_t = loadb(bl0, nbl0, "bl0", nc.gpsimd)
        bs1_b = cp.tile([128, OUT], F32, tag="bs1b", name="bs1b")
        nc.gpsimd.dma_start(out=bs1_b[:, :], in_=bs1b[:, :])
        bl1_b = cp.tile([128, OUT], F32, tag="bl1b", name="bl1b")
        nc.gpsimd.dma_start(out=bl1_b[:, :], in_=bl1b[:, :])
        naM = cp.tile([128, 1], F32, tag="naM", name="naM")
        nc.gpsimd.dma_start(out=naM[:, :], in_=naM_b[:, :])
        aH = cp.tile([128, 1], F32, tag="aH", name="aH")
        nc.gpsimd.dma_start(out=aH[:, :], in_=aH_b[:, :])
        ident32 = cp.tile([128, 128], F32, tag="i32", name="i32")
        nc.gpsimd.dma_start(out=ident32[:, :], in_=ident32_d[:, :])
        colmask = cp.tile([108, 1], F32, tag="cmask", name="cmask")
        nc.gpsimd.dma_start(out=colmask[:, :], in_=colmask_d[:])

        # ---------- persistent tiles ----------
        emb_loc = cp.tile([128, NT * OUT], F32, tag="emb_loc", name="emb_loc")
        tT_stack = cp.tile([64, P], F16, tag="tT_stack", name="tT_stack")
        e8pack_loc = cp.tile([128, NT * 128], F8, tag="e8l", name="e8l")
        maxs = cp.tile([128, 108], F32, tag="maxs", name="maxs")
        u_sb = cp.tile([1, OUT], F32, tag="u_sb", name="u_sb")
        wr_sb = cp.tile([64, 64], F32, tag="wr_sb", name="wr_sb")

        # ---------- generic fp16 MLP ----------
        def mlp16(XT_tiles, W0_t, b0_t, W1_t, b1_b, na_b, out_wide, pfx,
                  tile_cb=None):
            nh = HID // 128
            with tc.tile_pool(name=pfx + "_h", bufs=1) as hp, \
                 tc.tile_pool(name=pfx + "_r", bufs=2) as rp, \
                 tc.tile_pool(name=pfx + "_ps1", bufs=2, space="PSUM") as ps1, \
                 tc.tile_pool(name=pfx + "_ps2", bufs=2, space="PSUM") as ps2:
                h_tiles = [hp.tile([128, P], F16, tag=f"h{hs}",
                                   name=f"{pfx}h{hs}") for hs in range(nh)]
                for ns in range(2):
                    for hs in range(nh):
                        sl = slice(384 * ns, 384 * (ns + 1))
                        pp = ps1.tile([128, 384], F32, tag="l1", name="l1")
                        nf = len(XT_tiles)
                        for fc in range(nf):
                            nc.tensor.matmul(
                                pp[:, :],
                                W0_t[fc][:, 128 * hs:128 * (hs + 1)],
                                XT_tiles[fc][:, sl],
                                start=(fc == 0), stop=(fc == nf - 1))
                        r1 = rp.tile([128, 384], F32, tag="r1", name="r1")
                        nc.scalar.activation(r1[:, :], pp[:, :], AF.Relu,
                                             bias=b0_t[hs][0][:, :], scale=1.0)
                        r2 = rp.tile([128, 384], F32, tag="r2", name="r2")
                        nc.scalar.activation(r2[:, :], pp[:, :], AF.Relu,
                                             bias=b0_t[hs][1][:, :], scale=-1.0)
                        nc.vector.scalar_tensor_tensor(
                            h_tiles[hs][:, sl], r2[:, :], na_b[:, :],
                            r1[:, :], op0=OP.mult, op1=OP.add)
                for i in range(NT):
                    pp = ps2.tile([128, OUT], F32, tag="l2", name="l2")
                    for hs in range(nh):
                        nc.tensor.matmul(
                            pp[:, :],
                            h_tiles[hs][:, 128 * i:128 * (i + 1)],
                            W1_t[hs][:, :],
                            start=(hs == 0), stop=(hs == nh - 1))
                    nc.vector.tensor_add(out_wide[:, esl(i)], pp[:, :],
                                         b1_b[:, :])
                    if tile_cb is not None:
                        tile_cb(i)

        # ---------- hete MLP with fused per-tile exp chain ----------
        # t = softmax(e)/||softmax(e)|| = exp(e-max)/||exp(e-max)|| -- the
        # softmax divide cancels, and the rsqrt is batched after the loop
        # (per-tile Sqrt would thrash the activation table, 1.3us/reload).
        with tc.tile_pool(name="smax", bufs=2) as sp, \
             tc.tile_pool(name="ps_wu", bufs=1, space="PSUM") as pswu, \
             tc.tile_pool(name="ps_ttr", bufs=2, space="PSUM") as pstr:
            ps_wr = pswu.tile([64, 64], F32, tag="pswr", name="pswr")
            ps_u = pswu.tile([1, OUT], F32, tag="psu", name="psu")
            v_w = cp.tile([128, NT * OUT], F32, tag="v_w", name="v_w")
            e16_w = cp.tile([128, NT * OUT], F16, tag="e16w", name="e16w")
            dsq = cp.tile([128, NT], F32, tag="dsq", name="dsq")
            vsq = cp.tile([128, OUT], F32, tag="vsq", name="vsq")

            def hete_tile_cb(i):
                e = emb_loc[:, esl(i)]
                rmx = sp.tile([128, 1], F32, tag="rmx", name="rmx")
                nc.vector.tensor_reduce(rmx[:, :], e, axis=AX.X, op=OP.max,
                                        negate=True)
                v = v_w[:, esl(i)]
                nc.scalar.activation(v, e, AF.Exp, bias=rmx[:, :], scale=1.0)
                nc.scalar.activation(vsq[:, :], v, AF.Square,
                                     accum_out=dsq[:, i:i + 1])
                nc.vector.tensor_copy(e16_w[:, esl(i)], e)
                # emb8 hi/lo packed [hi|lo] per tile for the acc matmuls
                hi8 = e8pack_loc[:, 128 * i:128 * i + 64]
                nc.vector.tensor_copy(hi8, e)
                nc.vector.tensor_sub(e8pack_loc[:, 128 * i + 64:128 * (i + 1)],
                                     e, hi8)

            mlp16(XT_sm, Wh0_t, bh0_t, Wh1_t, bh1_b, naH, emb_loc, "hete",
                  tile_cb=hete_tile_cb)
            # batched normalize + fp16 cast + transpose
            rdw = sp.tile([128, NT], F32, tag="rdw", name="rdw")
            nc.vector.reciprocal(rdw[:, :], dsq[:, :])
            isdw = sp.tile([128, NT], F32, tag="isdw", name="isdw")
            nc.scalar.activation(isdw[:, :], rdw[:, :], AF.Sqrt)
            for i in range(NT):
                t_i = sp.tile([128, OUT], F32, tag="t_i", name="t_i")
                nc.vector.tensor_scalar(t_i[:, :], v_w[:, esl(i)],
                                        isdw[:, i:i + 1], None, OP.mult)
                nc.tensor.matmul(ps_u[:, :], ones_col[:, :], t_i[:, :],
                                 start=(i == 0), stop=(i == NT - 1))
                hi = sp.tile([128, OUT], F16, tag="hi", name="hi")
                nc.vector.tensor_copy(hi[:, :], t_i[:, :])
                nc.tensor.matmul(ps_wr[:, :], hi[:, :], e16_w[:, esl(i)],
                                 start=(i == 0), stop=(i == NT - 1))
                csl = slice(128 * i, 128 * (i + 1))
                pt = pstr.tile([64, 128], F16, tag="ttr", name="ttr")
                nc.tensor.transpose(pt[:, :], hi[:, :], ident16[:, :])
                nc.scalar.copy(tT_stack[0:64, csl], pt[:, :])
            nc.scalar.copy(wr_sb[:, :], ps_wr[:, :])
            nc.scalar.copy(u_sb[:, :], ps_u[:, :])

        # ---------- pack + merged AllGather ----------
        nc.sync.dma_start(out=agc_in[0:1, OFF_T:OFF_T + 24576],
                          in_=tT_stack[:, :].bitcast(F32))
        nc.scalar.dma_start(out=agc_in[0:1, OFF_E:OFF_E + 24576],
                            in_=e8pack_loc[:, :].bitcast(F32))
        nc.sync.dma_start(out=agc_in[0:1, OFF_W:OFF_W + 4096],
                          in_=wr_sb[:, :])
        nc.scalar.dma_start(out=agc_in[0:1, OFF_U:OFF_U + OUT],
                            in_=u_sb[:, :])
        nc.gpsimd.collective_compute(
            "AllGather", OP.bypass,
            ins=[agc_in[:, :]],
            outs=[agc_out[:, :]],
            replica_groups=[list(range(NCORES))])

        # ---------- fill the AG window ----------
        # own-block max tiles (diag suppressed), raw maxes
        with tc.tile_pool(name="ps_rex", bufs=2, space="PSUM") as psre0:
            for s in range(NT):
                for h in range(2):
                    sl = slice(384 * h, 384 * (h + 1))
                    pp = psre0.tile([128, 384], F32, tag="re", name="rex")
                    nc.tensor.matmul(pp[:, :],
                                     tT_stack[:, 128 * s:128 * (s + 1)],
                                     tT_stack[:, sl], start=True, stop=True)
                    if (s // 3) == h:
                        off = 128 * s - 384 * h
                        nc.vector.scalar_tensor_tensor(
                            pp[:, off:off + 128], ident32[:, :], -BIG,
                            pp[:, off:off + 128], op0=OP.mult, op1=OP.add)
                    slot = 96 + 2 * s + h
                    nc.vector.tensor_reduce(maxs[:, slot:slot + 1], pp[:, :],
                                            axis=AX.X, op=OP.max)

        # smooth + ori MLPs
        with tc.tile_pool(name="mlpout", bufs=1) as mo:
            sm_out = mo.tile([128, NT * OUT], F32, tag="smo", name="smo")

            def sm_cb(i):
                nc.scalar.dma_start(out=out_smooth[128 * i:128 * (i + 1), :],
                                    in_=sm_out[:, esl(i)])

            mlp16(XT_sm, Ws0_t, bs0_t, Ws1_t, bs1_b, naM, sm_out, "smooth",
                  tile_cb=sm_cb)

            or_out = mo.tile([128, NT * OUT], F32, tag="oro", name="oro")

            def or_cb(i):
                nc.scalar.dma_start(out=out_ori[128 * i:128 * (i + 1), :],
                                    in_=or_out[:, esl(i)])

            mlp16([XT_ori], Wl0_t, bl0_t, Wl1_t, bl1_b, naM, or_out, "ori",
                  tile_cb=or_cb)

        # ---------- unpack ----------
        tf_blk = [cp.tile([64, P], F16, tag=f"tf_{k}", name=f"tf_{k}")
                  for k in range(NCORES)]
        e8pack = [cp.tile([128, NT * 128], F8, tag=f"e8_{k}", name=f"e8_{k}")
                  for k in range(NCORES)]
        mb = cp.tile([128, 1], F32, tag="mb", name="mb")
        nmb = cp.tile([128, 1], F32, tag="nmb", name="nmb")
        nm256 = cp.tile([128, 1], F32, tag="nm256", name="nm256")
        w_sb = cp.tile([64, 64], F32, tag="w_sb", name="w_sb")
        U_sb = cp.tile([1, OUT], F32, tag="U_sb", name="U_sb")
        m01 = cp.tile([1, 1], F32, tag="m01", name="m01")

        with tc.tile_pool(name="unpack", bufs=1) as up:
            # m chain first (tiny, unlocks q8 bias)
            usum = up.tile([1, NCORES * OUT], F32, tag="usum", name="usum")
            nc.sync.dma_start(out=usum[:, :],
                              in_=agc_out[0:NCORES, OFF_U:OFF_U + OUT])
            nc.vector.tensor_reduce(
                U_sb[:, :],
                usum[:, :].rearrange("a (k o) -> a o k", k=NCORES),
                axis=AX.X, op=OP.add)
            usq = up.tile([1, OUT], F32, tag="usq", name="usq")
            uu = up.tile([1, 1], F32, tag="uu", name="uu")
            nc.scalar.activation(usq[:, :], U_sb[:, :], AF.Square,
                                 accum_out=uu[:, :])
            nc.vector.tensor_scalar(m01[:, :], uu[:, :], -float(N), INV_N2,
                                    OP.add, OP.mult)
            nc.gpsimd.partition_broadcast(mb[:, :], m01[:, :])
            nc.vector.tensor_scalar(nmb[:, :], mb[:, :], -1.0, None, OP.mult)
            nc.vector.tensor_scalar(nm256[:, :], mb[:, :], -QS, None, OP.mult)
            # bulk unpack, all on the sync queue in consumption order
            for k in range(NCORES):
                nc.sync.dma_start(out=tf_blk[k][:, :].bitcast(F32),
                                  in_=agc_out[k:k + 1, OFF_T:OFF_T + 24576])
                nc.sync.dma_start(out=e8pack[k][:, :].bitcast(F32),
                                  in_=agc_out[k:k + 1, OFF_E:OFF_E + 24576])
            # w sum (needed only post-relation, for tw)
            wsum = up.tile([64, NCORES * 64], F32, tag="wsum", name="wsum")
            for k in range(NCORES):
                nc.sync.dma_start(
                    out=wsum[:, 64 * k:64 * (k + 1)],
                    in_=agc_out[k:k + 1, OFF_W:OFF_W + 4096])
            nc.vector.tensor_reduce(
                w_sb[:, :],
                wsum[:, :].rearrange("p (k o) -> p o k", k=NCORES),
                axis=AX.X, op=OP.add)

        # ---------- fused relation + propagation pass ----------
        qeT16 = cp.tile([128, P], F16, tag="qeT", name="qeT")
        ceT16 = cp.tile([128, P], F16, tag="ceT", name="ceT")
        with tc.tile_pool(name="ps_acc", bufs=1, space="PSUM") as pacc:
            qe_ps = [pacc.tile([128, 384], F32, tag=f"qe{h}", name=f"qe{h}")
                     for h in range(2)]
            ce_ps = [pacc.tile([128, 384], F32, tag=f"ce{h}", name=f"ce{h}")
                     for h in range(2)]
            with tc.tile_pool(name="ps_re", bufs=4, space="PSUM") as psre:
                for k in range(NCORES):
                    for pr in range(NT // 2):
                        q8p = [qcp.tile([128, 768], F8, tag=f"q8p{h}",
                                        name=f"q8p{h}") for h in range(2)]
                        c8p = [qcp.tile([128, 768], F8, tag=f"c8p{h}",
                                        name=f"c8p{h}") for h in range(2)]
                        for d in range(2):
                            sub = 2 * pr + d
                            j = NT * k + sub
                            stk = tf_blk[k][:, 128 * sub:128 * (sub + 1)]
                            for h in range(2):
                                sl = slice(384 * h, 384 * (h + 1))
                                dsl = slice(384 * d, 384 * (d + 1))
                                pp = psre.tile([128, 384], F32, tag="re",
                                               name="rem")
                                nc.tensor.matmul(pp[:, :], stk,
                                                 tT_stack[:, sl],
                                                 start=True, stop=True)
                                nc.scalar.activation(q8p[h][:, dsl], pp[:, :],
                                                     AF.Relu,
                                                     bias=nm256[:, :],
                                                     scale=QS)
                                slot = 2 * j + h
                                nc.vector.tensor_reduce(
                                    maxs[:, slot:slot + 1], pp[:, :],
                                    axis=AX.X, op=OP.max)
                        # c = [q8 > 0] from the scaled fp8 tiles
                        for h in range(2):
                            nc.vector.tensor_scalar(
                                c8p[h][:, :], q8p[h][:, :], 1e-4, None,
                                OP.is_ge)
                        first = (k == 0 and pr == 0)
                        last = (k == NCORES - 1 and pr == NT // 2 - 1)
                        lhs8 = e8pack[k][:, 256 * pr:256 * (pr + 1)].rearrange(
                            "a (two m) -> a two m", two=2)
                        for h in range(2):
                            rq = q8p[h][:, :].rearrange("a (two n) -> a two n",
                                                        two=2)
                            rc = c8p[h][:, :].rearrange("a (two n) -> a two n",
                                                        two=2)
                            nc.tensor.matmul(
                                qe_ps[h][:, :], lhs8, rq,
                                start=first, stop=last,
                                perf_mode=PM.DoubleRow,
                                skip_group_check=True)
                            nc.tensor.matmul(
                                ce_ps[h][:, :], lhs8, rc,
                                start=first, stop=last,
                                perf_mode=PM.DoubleRow,
                                skip_group_check=True)
            # qe/ce out of PSUM (fp16) so the psre pool can close
            for h in range(2):
                sl = slice(384 * h, 384 * (h + 1))
                nc.scalar.copy(qeT16[:, sl], qe_ps[h][:, :])
                nc.scalar.copy(ceT16[:, sl], ce_ps[h][:, :])

        # ---------- max stat -> AG2 ----------
        with tc.tile_pool(name="stats", bufs=1) as stp, \
             tc.tile_pool(name="ps_st", bufs=1, space="PSUM") as psst:
            ptm = psst.tile([108, 128], F32, tag="mtr", name="mtr")
            nc.tensor.transpose(ptm[:, :], maxs[:, :], ident32[:, :])
            mm_ = stp.tile([108, 128], F32, tag="mm", name="mm")
            nc.vector.tensor_scalar(mm_[:, :], ptm[:, :], colmask[:, :],
                                    None, OP.mult)
            mv = stp.tile([108, 1], F32, tag="mv", name="mv")
            nc.vector.tensor_reduce(mv[:, :], mm_[:, :], axis=AX.X,
                                    op=OP.max)
            ptm2 = psst.tile([1, 108], F32, tag="mtr2", name="mtr2")
            nc.tensor.transpose(ptm2[:, :], mv[:, :],
                                ident32[0:108, 0:108])
            mrow = stp.tile([1, 108], F32, tag="mrow", name="mrow")
            nc.scalar.copy(mrow[:, :], ptm2[:, :])
            mx01 = stp.tile([1, 1], F32, tag="mx01", name="mx01")
            nc.vector.tensor_reduce(mx01[:, :], mrow[:, :], axis=AX.X,
                                    op=OP.max)
            nc.sync.dma_start(out=ag2_in[0:1, 0:1], in_=mx01[:, :])
        nc.gpsimd.collective_compute(
            "AllGather", OP.bypass,
            ins=[ag2_in[:, :]],
            outs=[ag2_out[:, :]],
            replica_groups=[list(range(NCORES))])

        # ---------- AG2-window fill: tw, transposes, pre-ip epilogue ----
        if True:
            with tc.tile_pool(name="epi", bufs=1) as ep, \
                 tc.tile_pool(name="ps_epi", bufs=2, space="PSUM") as pse:
                # scalars derived from m
                imb = cp.tile([128, 1], F32, tag="imb", name="imb")
                nimb = cp.tile([128, 1], F32, tag="nimb", name="nimb")
                n1m = cp.tile([128, 1], F32, tag="n1m", name="n1m")
                nc.vector.reciprocal(imb[:, :], mb[:, :])
                nc.vector.tensor_scalar(nimb[:, :], imb[:, :], -1.0, None,
                                        OP.mult)
                nc.vector.tensor_scalar(n1m[:, :], mb[:, :], 1.0, -1.0,
                                        OP.mult, OP.add)
                # tw = t @ w  (fp16 t-hi)
                w16 = ep.tile([64, 64], F16, tag="w16", name="w16")
                nc.vector.tensor_copy(w16[:, :], w_sb[:, :])
                tw_nm = ep.tile([128, NT * OUT], F32, tag="tw", name="tw")
                for i in range(NT):
                    ptw = pse.tile([128, OUT], F32, tag="ptw", name="ptw")
                    nc.tensor.matmul(ptw[:, :],
                                     tT_stack[:, 128 * i:128 * (i + 1)],
                                     w16[:, :], start=True, stop=True)
                    nc.scalar.copy(tw_nm[:, esl(i)], ptw[:, :])
                # qe/ce to node-major via fp16 transposes; hi+lo plane add.
                # identS = I/256 folds the q8 unscale into the qe transpose.
                qe_nm = ep.tile([128, NT * OUT], F32, tag="qe_nm",
                                name="qe_nm")
                ce_nm = ep.tile([128, NT * OUT], F32, tag="ce_nm",
                                name="ce_nm")
                for i in range(NT):
                    csl = slice(128 * i, 128 * (i + 1))
                    pq = pse.tile([128, 128], F16, tag="tq", name="tq")
                    nc.tensor.transpose(pq[:, :], qeT16[:, csl],
                                        identS[:, :])
                    qtmp = ep.tile([128, 128], F16, tag="qtmp", name="qtmp")
                    nc.scalar.copy(qtmp[:, :], pq[:, :])
                    nc.vector.tensor_add(qe_nm[:, esl(i)], qtmp[:, 0:64],
                                         qtmp[:, 64:128])
                    pc = pse.tile([128, 128], F16, tag="tc", name="tc")
                    nc.tensor.transpose(pc[:, :], ceT16[:, csl],
                                        ident16[:, :])
                    ctmp = ep.tile([128, 128], F16, tag="ctmp", name="ctmp")
                    nc.scalar.copy(ctmp[:, :], pc[:, :])
                    nc.vector.tensor_add(ce_nm[:, esl(i)], ctmp[:, 0:64],
                                         ctmp[:, 64:128])
                # G = (qe - tw)/m + ce ;  z1 = qe + (m-1)*emb
                G = ep.tile([128, NT * OUT], F32, tag="G", name="G")
                nc.vector.scalar_tensor_tensor(G[:, :], qe_nm[:, :],
                                               imb[:, :], ce_nm[:, :],
                                               op0=OP.mult, op1=OP.add)
                nc.vector.scalar_tensor_tensor(G[:, :], tw_nm[:, :],
                                               nimb[:, :], G[:, :],
                                               op0=OP.mult, op1=OP.add)
                z1 = ep.tile([128, NT * OUT], F32, tag="z1", name="z1")
                nc.vector.scalar_tensor_tensor(z1[:, :], emb_loc[:, :],
                                               n1m[:, :], qe_nm[:, :],
                                               op0=OP.mult, op1=OP.add)
                cpos = ep.tile([128, NT * OUT], F32, tag="cpos", name="cpos")
                nc.vector.scalar_tensor_tensor(cpos[:, :], G[:, :], aH[:, :],
                                               emb_loc[:, :], op0=OP.mult,
                                               op1=OP.add)
                cneg = ep.tile([128, NT * OUT], F32, tag="cneg", name="cneg")
                nc.vector.scalar_tensor_tensor(cneg[:, :], emb_loc[:, :],
                                               naH[:, :], G[:, :],
                                               op0=OP.mult, op1=OP.subtract)
                emb_half = ep.tile([128, NT * OUT], F32, tag="ehalf",
                                   name="ehalf")
                nc.vector.tensor_scalar(emb_half[:, :], emb_loc[:, :], 0.5,
                                        None, OP.mult)
                # pre-warm the Exp activation table during the AG2 window
                dmx = ep.tile([1, 1], F32, tag="dmx", name="dmx")
                nc.scalar.activation(dmx[:, :], m01[:, :], AF.Exp)

                # ---------- AG2 -> ip ----------
                ipb = cp.tile([128, 1], F32, tag="ipb", name="ipb")
                naip = cp.tile([128, 1], F32, tag="naip", name="naip")
                with tc.tile_pool(name="glob", bufs=1) as gp:
                    m8 = gp.tile([1, NCORES], F32, tag="m8", name="m8")
                    nc.sync.dma_start(out=m8[:, :], in_=ag2_out[:, 0:1])
                    mxg = gp.tile([1, 1], F32, tag="mxg", name="mxg")
                    nc.vector.tensor_reduce(mxg[:, :], m8[:, :], axis=AX.X,
                                            op=OP.max)
                    pd = gp.tile([1, 1], F32, tag="pd", name="pd")
                    nc.vector.tensor_sub(pd[:, :], mxg[:, :], m01[:, :])
                    ip01 = gp.tile([1, 1], F32, tag="ip01", name="ip01")
                    nc.vector.reciprocal(ip01[:, :], pd[:, :])
                    nc.gpsimd.partition_broadcast(ipb[:, :], ip01[:, :])
                    nc.vector.tensor_mul(naip[:, :], ipb[:, :], naH[:, :])

                pos_w = ep.tile([128, NT * OUT], F32, tag="pos_w",
                                name="pos_w")
                nc.vector.scalar_tensor_tensor(pos_w[:, :], z1[:, :],
                                               ipb[:, :], cpos[:, :],
                                               op0=OP.mult, op1=OP.add)
                neg_w = ep.tile([128, NT * OUT], F32, tag="neg_w",
                                name="neg_w")
                nc.vector.scalar_tensor_tensor(neg_w[:, :], z1[:, :],
                                               naip[:, :], cneg[:, :],
                                               op0=OP.mult, op1=OP.add)

                pp_w = ep.tile([128, NT * OUT], F32, tag="pp_w", name="pp_w")
                pn_w = ep.tile([128, NT * OUT], F32, tag="pn_w", name="pn_w")
                for src_w, dst in ((pos_w, pp_w), (neg_w, pn_w)):
                    rmx2 = ep.tile([128, NT], F32, tag="rmx2", name="rmx2")
                    nc.vector.tensor_reduce(
                        rmx2[:, :],
                        src_w[:, :].rearrange("p (g o) -> p g o", o=OUT),
                        axis=AX.X, op=OP.max, negate=True)
                    ex2 = ep.tile([128, NT * OUT], F32, tag="ex2", name="ex2")
                    ssum2 = ep.tile([128, NT], F32, tag="ssum2", name="ssum2")
                    for i in range(NT):
                        nc.scalar.activation(ex2[:, esl(i)], src_w[:, esl(i)],
                                             AF.Exp, bias=rmx2[:, i:i + 1],
                                             scale=1.0)
                    nc.vector.tensor_reduce(
                        ssum2[:, :],
                        ex2[:, :].rearrange("p (g o) -> p g o", o=OUT),
                        axis=AX.X, op=OP.add)
                    rs2 = ep.tile([128, NT], F32, tag="rs2", name="rs2")
                    nc.vector.reciprocal(rs2[:, :], ssum2[:, :])
                    for i in range(NT):
                        nc.vector.tensor_scalar(dst[:, esl(i)], ex2[:, esl(i)],
                                                rs2[:, i:i + 1], None, OP.mult)
                dd = ep.tile([128, NT * OUT], F32, tag="dd", name="dd")
                nc.vector.scalar_tensor_tensor(dd[:, :], pp_w[:, :], 0.5,
                                               emb_half[:, :], op0=OP.mult,
                                               op1=OP.add)
                msg = ep.tile([128, NT * OUT], F32, tag="msg", name="msg")
                nc.vector.scalar_tensor_tensor(msg[:, :], pn_w[:, :], -0.5,
                                               dd[:, :], op0=OP.mult,
                                               op1=OP.add)
                for i in range(NT):
                    eng = (nc.gpsimd, nc.sync, nc.scalar)[i % 3]
                    eng.dma_start(out=out_msg[128 * i:128 * (i + 1), :],
                                  in_=msg[:, esl(i)])

    nc.compile()
    return nc


def _get_nc():
    if "nc" not in _CACHE:
        _CACHE["nc"] = _build()
    return _CACHE["nc"]


def _make_in_maps(inputs):
    f = np.float32
    h = np.float16
    sm = np.ascontiguousarray(inputs["smoothed_feature"], dtype=f)
    ori = np.ascontiguousarray(inputs["ori_feature"], dtype=f)
    aM = float(np.asarray(inputs["prelu_model"]).reshape(-1)[0])
    aHv = float(np.asarray(inputs["prelu_hete"]).reshape(-1)[0])

    def b0pair(name):
        b = np.ascontiguousarray(inputs[name], dtype=f)
        return b, np.ascontiguousarray(-b, dtype=f)

    bh0v, nbh0v = b0pair("b_hete0")
    bs0v, nbs0v = b0pair("b_smooth0")
    bl0v, nbl0v = b0pair("b_local0")

    def b1b(name):
        return np.ascontiguousarray(
            np.broadcast_to(np.asarray(inputs[name], dtype=f), (128, OUT)))

    shared = {
        "Wh0": inputs["W_hete0"].astype(h), "Wh1": inputs["W_hete1"].astype(h),
        "Ws0": inputs["W_smooth0"].astype(h),
        "Ws1": inputs["W_smooth1"].astype(h),
        "Wl0": inputs["W_local0"].astype(h),
        "Wl1": inputs["W_local1"].astype(h),
        "bh0": bh0v, "nbh0": nbh0v, "bs0": bs0v, "nbs0": nbs0v,
        "bl0": bl0v, "nbl0": nbl0v,
        "bh1b": b1b("b_hete1"), "bs1b": b1b("b_smooth1"),
        "bl1b": b1b("b_local1"),
        "naH_b": np.full((128, 1), -aHv, dtype=f),
        "naM_b": np.full((128, 1), -aM, dtype=f),
        "aH_b": np.full((128, 1), aHv, dtype=f),
        "ident16": np.eye(128, dtype=h),
        "identS": (np.eye(128) / QS).astype(h),
        "ident32": np.eye(128, dtype=f),
        "ones_col": np.ones((128, 1), dtype=f),
    }
    shared = {k: np.ascontiguousarray(v) for k, v in shared.items()}
    in_maps = []
    for r in range(NCORES):
        cm = np.ones(108, dtype=f)
        cm[2 * NT * r:2 * NT * (r + 1)] = 0.0  # drop raw own-block slots
        m = dict(shared)
        m["xT_sm"] = np.ascontiguousarray(
            sm[P * r:P * (r + 1)].T.astype(h))
        m["xT_ori"] = np.ascontiguousarray(
            ori[P * r:P * (r + 1)].T.astype(h))
        m["colmask"] = cm
        in_maps.append(m)
    return in_maps


def _ensure_ntff_hook():
    """The agent image's antenv lacks axon_hooks; shim it so
    run_bass_kernel_spmd(trace=True) can capture NTFF profiles."""
    if "antenv.axon_hooks" in sys.modules:
        return
    import types
    import antenv
    mod = types.ModuleType("antenv.axon_hooks")
    state = {"hook": None}
    mod.set_axon_ntff_profile_hook = lambda h: state.__setitem__("hook", h)
    mod.get_axon_ntff_profile_hook = lambda: state["hook"]
    sys.modules["antenv.axon_hooks"] = mod
    antenv.axon_hooks = mod
    try:
        from trn_agent_boot.trn_boot import _ntff_profile_via_ctypes
        mod.set_axon_ntff_profile_hook(
            _ntff_profile_via_ctypes("/opt/axon/libaxon_pjrt.so"))
    except Exception as e:
        print(f"ntff hook install failed: {e}", file=sys.stderr)


def run(inputs, trace=False):
    if trace:
        _ensure_ntff_hook()
    nc = _get_nc()
    in_maps = _make_in_maps(inputs)
    res = run_bass_kernel_spmd(nc, in_maps, list(range(NCORES)), trace=trace)
    outs = res.results
    o1 = np.concatenate([outs[r]["out_ori"] for r in range(NCORES)], axis=0)
    o2 = np.concatenate([outs[r]["out_smooth"] for r in range(NCORES)], axis=0)
    o3 = np.concatenate([outs[r]["out_msg"] for r in range(NCORES)], axis=0)
    return (o1.astype(np.float32), o2.astype(np.float32),
            o3.astype(np.float32)), res


def kernel(**inputs):
    (o1, o2, o3), _ = run(inputs, trace=False)
    return (o1, o2, o3)


# revision 6
# speedup vs baseline: 3.2690x; 1.0023x over previous
"""AdaFGL Bass kernel for 8 TRN2 NeuronCores (v3).

Row-shards N=6144 nodes across 8 cores (768 each). Design vs v2:

- re computed from t-hi fp16 ONLY (the lo-correction pass is dropped;
  measured end-to-end l2 5.1e-3 vs the 2e-2 gate).  Halves the re
  matmul work and the tT share of the AG payload.
- q stored as fp8(256*relu(re-m)): the x256 scale moves small q out of
  the fp8e4m3 subnormal range, so c = [re>=m] becomes recoverable as
  is_ge(q8,eps) FROM THE fp8 TILE.  The c pass therefore reads SBUF
  (not PSUM) and runs on the otherwise-idle GpSimd engine, one
  [128,768] op per (iter,h).  The 1/256 unscale is folded into the
  qe transpose identity (I/256), so the epilogue is unchanged.
- With c uniform {0,1} (no Sign halves) the whole Ue machinery
  (colsum(emb), the (se+Ue)/2 fold, the AG2 ue payload + broadcasts)
  is deleted; AG2 carries only mx.
- emb fp8 hi/lo planes are PACKED [hi|lo] per tile into one e8pack
  tile, used as a 256-wide stationary in the DoubleRow acc matmuls:
  qe (and ce) vs BOTH planes in ONE pass each -> 4 acc matmuls/iter
  instead of 8.  The hi+lo combine rides the existing epilogue
  transposes ([128,128] chunks, then one [128,64] add per tile).
- Relation-pass engine split: ACT 4x relu-q8, DVE 4x max-reduce,
  GpSimd 2x c-derive; PE 4 re + 4 acc matmuls per iter.
- A dummy 64B AllGather issues at t~0 so the runtime's first-collective
  barrier (39us!) runs in the shadow of the hete MLP instead of
  delaying AG1.
"""

import sys, os
sys.path.insert(0, "/opt/trn_rl_repo")

import numpy as np
from contextlib import ExitStack

from concourse import bass, bacc, tile, mybir
from concourse.bass_utils import run_bass_kernel_spmd

F32 = mybir.dt.float32
F16 = mybir.dt.float16
F8 = mybir.dt.float8e4
AX = mybir.AxisListType
OP = mybir.AluOpType
AF = mybir.ActivationFunctionType
PM = mybir.MatmulPerfMode

N = 6144
NCORES = 8
P = N // NCORES            # 768 rows per core
FEAT = 128
INSM = 512
HID = 256
OUT = 64
NT = P // 128              # 6 row tiles per core
NJ = N // 128              # 48 column chunks
INV_N2 = 1.0 / float(N * N)
BIG = 1.0e6
QS = 256.0                 # q8 scale (fp8e4m3 max 448 >> 256*max(q))

# merged AG payload layout (f32 words)
OFF_T = 0                  # t hi fp16 [64, 768] -> 24576 w
OFF_E = 24576              # e8pack fp8 [128, 768] -> 24576 w
OFF_W = OFF_E + 24576      # wr f32 [64, 64] -> 4096 w
OFF_U = OFF_W + 4096       # u f32 [1, 64]
AGW = OFF_U + 64

_CACHE = {}


def _build():
    nc = bacc.Bacc("TRN2", target_bir_lowering=False, debug=False,
                   num_devices=NCORES)

    def din(name, shape, dt=F32):
        return nc.dram_tensor(name, list(shape), dt, kind="ExternalInput").ap()

    def dout(name, shape):
        return nc.dram_tensor(name, list(shape), F32, kind="ExternalOutput").ap()

    xT_sm = din("xT_sm", (INSM, P), F16)
    xT_ori = din("xT_ori", (FEAT, P), F16)
    Wh0 = din("Wh0", (INSM, HID), F16); Wh1 = din("Wh1", (HID, OUT), F16)
    Ws0 = din("Ws0", (INSM, HID), F16); Ws1 = din("Ws1", (HID, OUT), F16)
    Wl0 = din("Wl0", (FEAT, HID), F16); Wl1 = din("Wl1", (HID, OUT), F16)
    bh0 = din("bh0", (HID,)); nbh0 = din("nbh0", (HID,))
    bs0 = din("bs0", (HID,)); nbs0 = din("nbs0", (HID,))
    bl0 = din("bl0", (HID,)); nbl0 = din("nbl0", (HID,))
    bh1b = din("bh1b", (128, OUT))
    bs1b = din("bs1b", (128, OUT))
    bl1b = din("bl1b", (128, OUT))
    naH_b = din("naH_b", (128, 1))
    naM_b = din("naM_b", (128, 1))
    aH_b = din("aH_b", (128, 1))
    ident16_d = din("ident16", (128, 128), F16)
    identS_d = din("identS", (128, 128), F16)   # I/256 for qe unscale
    ident32_d = din("ident32", (128, 128))
    ones_col_d = din("ones_col", (128, 1))
    colmask_d = din("colmask", (108,))

    out_ori = dout("out_ori", (P, OUT))
    out_smooth = dout("out_smooth", (P, OUT))
    out_msg = dout("out_msg", (P, OUT))

    agc_in = nc.dram_tensor("agc_in", [1, AGW], F32).ap()
    agc_out = nc.dram_tensor("agc_out", [NCORES, AGW], F32,
                             addr_space="Shared").ap()
    ag2_in = nc.dram_tensor("ag2_in", [1, 16], F32).ap()
    ag2_out = nc.dram_tensor("ag2_out", [NCORES, 16], F32,
                             addr_space="Shared").ap()
    agd_in = nc.dram_tensor("agd_in", [1, 16], F32).ap()
    agd_out = nc.dram_tensor("agd_out", [NCORES, 16], F32,
                             addr_space="Shared").ap()

    def esl(i):
        return slice(OUT * i, OUT * (i + 1))

    with tile.TileContext(nc) as tc, ExitStack() as ctx:
        # dummy collective first: absorbs the runtime's first-collective
        # barrier into the MLP window
        nc.gpsimd.collective_compute(
            "AllGather", OP.bypass,
            ins=[agd_in[:, :]],
            outs=[agd_out[:, :]],
            replica_groups=[list(range(NCORES))])

        cp = ctx.enter_context(tc.tile_pool(name="const", bufs=1))
        qcp = ctx.enter_context(tc.tile_pool(name="qc", bufs=3))

        # ---------- load constants / weights (hete path first) ----------
        def loadw(dram_ap, rows, cols, tag, eng):
            tiles = []
            for i in range(rows // 128):
                t = cp.tile([128, cols], F16, tag=f"{tag}_{i}",
                            name=f"{tag}_{i}")
                eng.dma_start(out=t[:, :],
                              in_=dram_ap[128 * i:128 * (i + 1), :])
                tiles.append(t)
            return tiles

        def loadb(dram_ap, ndram_ap, tag, eng):
            tiles = []
            for i in range(HID // 128):
                t = cp.tile([128, 1], F32, tag=f"{tag}_{i}", name=f"{tag}_{i}")
                eng.dma_start(out=t[:, :],
                              in_=dram_ap[128 * i:128 * (i + 1)])
                n = cp.tile([128, 1], F32, tag=f"{tag}n_{i}",
                            name=f"{tag}n_{i}")
                eng.dma_start(out=n[:, :],
                              in_=ndram_ap[128 * i:128 * (i + 1)])
                tiles.append((t, n))
            return tiles

        # x first on sync queue so the hete MLP can start ASAP
        XT_sm = []
        for f in range(INSM // 128):
            t = cp.tile([128, P], F16, tag=f"xts_{f}", name=f"xts_{f}")
            nc.sync.dma_start(out=t[:, :],
                              in_=xT_sm[128 * f:128 * (f + 1), :])
            XT_sm.append(t)
        Wh0_t = loadw(Wh0, INSM, HID, "Wh0", nc.scalar)
        Wh1_t = loadw(Wh1, HID, OUT, "Wh1", nc.scalar)
        bh0_t = loadb(bh0, nbh0, "bh0", nc.sync)
        bh1_b = cp.tile([128, OUT], F32, tag="bh1b", name="bh1b")
        nc.sync.dma_start(out=bh1_b[:, :], in_=bh1b[:, :])
        naH = cp.tile([128, 1], F32, tag="naH", name="naH")
        nc.sync.dma_start(out=naH[:, :], in_=naH_b[:, :])
        ident16 = cp.tile([128, 128], F16, tag="i16", name="i16")
        nc.sync.dma_start(out=ident16[:, :], in_=ident16_d[:, :])
        identS = cp.tile([128, 128], F16, tag="iS", name="iS")
        nc.sync.dma_start(out=identS[:, :], in_=identS_d[:, :])
        ones_col = cp.tile([128, 1], F32, tag="onec", name="onec")
        nc.sync.dma_start(out=ones_col[:, :], in_=ones_col_d[:, :])

        # fill-phase constants (gpsimd queue: idle until the AG trigger,
        # and these all land long before it)
        XT_ori = cp.tile([128, P], F16, tag="xto", name="xto")
        nc.gpsimd.dma_start(out=XT_ori[:, :], in_=xT_ori[:, :])
        Ws0_t = loadw(Ws0, INSM, HID, "Ws0", nc.gpsimd)
        Ws1_t = loadw(Ws1, HID, OUT, "Ws1", nc.gpsimd)
        Wl0_t = loadw(Wl0, FEAT, HID, "Wl0", nc.gpsimd)
        Wl1_t = loadw(Wl1, HID, OUT, "Wl1", nc.gpsimd)
        bs0_t = loadb(bs0, nbs0, "bs0", nc.gpsimd)
        bl0_t = loadb(bl0, nbl0, "bl0", nc.gpsimd)
        bs1_b = cp.tile([128, OUT], F32, tag="bs1b", name="bs1b")
        nc.gpsimd.dma_start(out=bs1_b[:, :], in_=bs1b[:, :])
        bl1_b = cp.tile([128, OUT], F32, tag="bl1b", name="bl1b")
        nc.gpsimd.dma_start(out=bl1_b[:, :], in_=bl1b[:, :])
        naM = cp.tile([128, 1], F32, tag="naM", name="naM")
        nc.gpsimd.dma_start(out=naM[:, :], in_=naM_b[:, :])
        aH = cp.tile([128, 1], F32, tag="aH", name="aH")
        nc.gpsimd.dma_start(out=aH[:, :], in_=aH_b[:, :])
        ident32 = cp.tile([128, 128], F32, tag="i32", name="i32")
        nc.gpsimd.dma_start(out=ident32[:, :], in_=ident32_d[:, :])
        colmask = cp.tile([108, 1], F32, tag="cmask", name="cmask")
        nc.gpsimd.dma_start(out=colmask[:, :], in_=colmask_d[:])

        # ---------- persistent tiles ----------
        emb_loc = cp.tile([128, NT * OUT], F32, tag="emb_loc", name="emb_loc")
        tT_stack = cp.tile([64, P], F16, tag="tT_stack", name="tT_stack")
        e8pack_loc = cp.tile([128, NT * 128], F8, tag="e8l", name="e8l")
        maxs = cp.tile([128, 108], F32, tag="maxs", name="maxs")
        u_sb = cp.tile([1, OUT], F32, tag="u_sb", name="u_sb")
        wr_sb = cp.tile([64, 64], F32, tag="wr_sb", name="wr_sb")

        # ---------- generic fp16 MLP ----------
        def mlp16(XT_tiles, W0_t, b0_t, W1_t, b1_b, na_b, out_wide, pfx,
                  tile_cb=None):
            nh = HID // 128
            with tc.tile_pool(name=pfx + "_h", bufs=1) as hp, \
                 tc.tile_pool(name=pfx + "_r", bufs=2) as rp, \
                 tc.tile_pool(name=pfx + "_ps1", bufs=2, space="PSUM") as ps1, \
                 tc.tile_pool(name=pfx + "_ps2", bufs=2, space="PSUM") as ps2:
                h_tiles = [hp.tile([128, P], F16, tag=f"h{hs}",
                                   name=f"{pfx}h{hs}") for hs in range(nh)]
                for ns in range(2):
                    for hs in range(nh):
                        sl = slice(384 * ns, 384 * (ns + 1))
                        pp = ps1.tile([128, 384], F32, tag="l1", name="l1")
                        nf = len(XT_tiles)
                        for fc in range(nf):
                            nc.tensor.matmul(
                                pp[:, :],
                                W0_t[fc][:, 128 * hs:128 * (hs + 1)],
                                XT_tiles[fc][:, sl],
                                start=(fc == 0), stop=(fc == nf - 1))
                        r1 = rp.tile([128, 384], F32, tag="r1", name="r1")
                        nc.scalar.activation(r1[:, :], pp[:, :], AF.Relu,
                                             bias=b0_t[hs][0][:, :], scale=1.0)
                        r2 = rp.tile([128, 384], F32, tag="r2", name="r2")
                        nc.scalar.activation(r2[:, :], pp[:, :], AF.Relu,
                                             bias=b0_t[hs][1][:, :], scale=-1.0)
                        nc.vector.scalar_tensor_tensor(
                            h_tiles[hs][:, sl], r2[:, :], na_b[:, :],
                            r1[:, :], op0=OP.mult, op1=OP.add)
                for i in range(NT):
                    pp = ps2.tile([128, OUT], F32, tag="l2", name="l2")
                    for hs in range(nh):
                        nc.tensor.matmul(
                            pp[:, :],
                            h_tiles[hs][:, 128 * i:128 * (i + 1)],
                            W1_t[hs][:, :],
                            start=(hs == 0), stop=(hs == nh - 1))
                    nc.vector.tensor_add(out_wide[:, esl(i)], pp[:, :],
                                         b1_b[:, :])
                    if tile_cb is not None:
                        tile_cb(i)

        # ---------- hete MLP with fused per-tile exp chain ----------
        # t = softmax(e)/||softmax(e)|| = exp(e-max)/||exp(e-max)|| -- the
        # softmax divide cancels, and the rsqrt is batched after the loop
        # (per-tile Sqrt would thrash the activation table, 1.3us/reload).
        with tc.tile_pool(name="smax", bufs=2) as sp, \
             tc.tile_pool(name="ps_wu", bufs=1, space="PSUM") as pswu, \
             tc.tile_pool(name="ps_ttr", bufs=2, space="PSUM") as pstr:
            ps_wr = pswu.tile([64, 64], F32, tag="pswr", name="pswr")
            ps_u = pswu.tile([1, OUT], F32, tag="psu", name="psu")
            v_w = cp.tile([128, NT * OUT], F32, tag="v_w", name="v_w")
            e16_w = cp.tile([128, NT * OUT], F16, tag="e16w", name="e16w")
            dsq = cp.tile([128, NT], F32, tag="dsq", name="dsq")
            vsq = cp.tile([128, OUT], F32, tag="vsq", name="vsq")

            def hete_tile_cb(i):
                e = emb_loc[:, esl(i)]
                rmx = sp.tile([128, 1], F32, tag="rmx", name="rmx")
                nc.vector.tensor_reduce(rmx[:, :], e, axis=AX.X, op=OP.max,
                                        negate=True)
                v = v_w[:, esl(i)]
                nc.scalar.activation(v, e, AF.Exp, bias=rmx[:, :], scale=1.0)
                nc.scalar.activation(vsq[:, :], v, AF.Square,
                                     accum_out=dsq[:, i:i + 1])
                nc.vector.tensor_copy(e16_w[:, esl(i)], e)
                # emb8 hi/lo packed [hi|lo] per tile for the acc matmuls
                hi8 = e8pack_loc[:, 128 * i:128 * i + 64]
                nc.vector.tensor_copy(hi8, e)
                nc.vector.tensor_sub(e8pack_loc[:, 128 * i + 64:128 * (i + 1)],
                                     e, hi8)

            mlp16(XT_sm, Wh0_t, bh0_t, Wh1_t, bh1_b, naH, emb_loc, "hete",
                  tile_cb=hete_tile_cb)
            # batched normalize + fp16 cast + transpose
            rdw = sp.tile([128, NT], F32, tag="rdw", name="rdw")
            nc.vector.reciprocal(rdw[:, :], dsq[:, :])
            isdw = sp.tile([128, NT], F32, tag="isdw", name="isdw")
            nc.scalar.activation(isdw[:, :], rdw[:, :], AF.Sqrt)
            for i in range(NT):
                t_i = sp.tile([128, OUT], F32, tag="t_i", name="t_i")
                nc.vector.tensor_scalar(t_i[:, :], v_w[:, esl(i)],
                                        isdw[:, i:i + 1], None, OP.mult)
                nc.tensor.matmul(ps_u[:, :], ones_col[:, :], t_i[:, :],
                                 start=(i == 0), stop=(i == NT - 1))
                hi = sp.tile([128, OUT], F16, tag="hi", name="hi")
                nc.vector.tensor_copy(hi[:, :], t_i[:, :])
                nc.tensor.matmul(ps_wr[:, :], hi[:, :], e16_w[:, esl(i)],
                                 start=(i == 0), stop=(i == NT - 1))
                csl = slice(128 * i, 128 * (i + 1))
                pt = pstr.tile([64, 128], F16, tag="ttr", name="ttr")
                nc.tensor.transpose(pt[:, :], hi[:, :], ident16[:, :])
                nc.scalar.copy(tT_stack[0:64, csl], pt[:, :])
            nc.scalar.copy(wr_sb[:, :], ps_wr[:, :])
            nc.scalar.copy(u_sb[:, :], ps_u[:, :])

        # ---------- pack + merged AllGather ----------
        nc.sync.dma_start(out=agc_in[0:1, OFF_T:OFF_T + 24576],
                          in_=tT_stack[:, :].bitcast(F32))
        nc.scalar.dma_start(out=agc_in[0:1, OFF_E:OFF_E + 24576],
                            in_=e8pack_loc[:, :].bitcast(F32))
        nc.sync.dma_start(out=agc_in[0:1, OFF_W:OFF_W + 4096],
                          in_=wr_sb[:, :])
        nc.scalar.dma_start(out=agc_in[0:1, OFF_U:OFF_U + OUT],
                            in_=u_sb[:, :])
        nc.gpsimd.collective_compute(
            "AllGather", OP.bypass,
            ins=[agc_in[:, :]],
            outs=[agc_out[:, :]],
            replica_groups=[list(range(NCORES))])

        # ---------- fill the AG window ----------
        # own-block max tiles (diag suppressed), raw maxes
        with tc.tile_pool(name="ps_rex", bufs=2, space="PSUM") as psre0:
            for s in range(NT):
                for h in range(2):
                    sl = slice(384 * h, 384 * (h + 1))
                    pp = psre0.tile([128, 384], F32, tag="re", name="rex")
                    nc.tensor.matmul(pp[:, :],
                                     tT_stack[:, 128 * s:128 * (s + 1)],
                                     tT_stack[:, sl], start=True, stop=True)
                    if (s // 3) == h:
                        off = 128 * s - 384 * h
                        nc.vector.scalar_tensor_tensor(
                            pp[:, off:off + 128], ident32[:, :], -BIG,
                            pp[:, off:off + 128], op0=OP.mult, op1=OP.add)
                    slot = 96 + 2 * s + h
                    nc.vector.tensor_reduce(maxs[:, slot:slot + 1], pp[:, :],
                                            axis=AX.X, op=OP.max)

        # smooth + ori MLPs
        with tc.tile_pool(name="mlpout", bufs=1) as mo:
            sm_out = mo.tile([128, NT * OUT], F32, tag="smo", name="smo")

            def sm_cb(i):
                nc.scalar.dma_start(out=out_smooth[128 * i:128 * (i + 1), :],
                                    in_=sm_out[:, esl(i)])

            mlp16(XT_sm, Ws0_t, bs0_t, Ws1_t, bs1_b, naM, sm_out, "smooth",
                  tile_cb=sm_cb)

            or_out = mo.tile([128, NT * OUT], F32, tag="oro", name="oro")

            def or_cb(i):
                nc.scalar.dma_start(out=out_ori[128 * i:128 * (i + 1), :],
                                    in_=or_out[:, esl(i)])

            mlp16([XT_ori], Wl0_t, bl0_t, Wl1_t, bl1_b, naM, or_out, "ori",
                  tile_cb=or_cb)

        # ---------- unpack ----------
        tf_blk = [cp.tile([64, P], F16, tag=f"tf_{k}", name=f"tf_{k}")
                  for k in range(NCORES)]
        e8pack = [cp.tile([128, NT * 128], F8, tag=f"e8_{k}", name=f"e8_{k}")
                  for k in range(NCORES)]
        mb = cp.tile([128, 1], F32, tag="mb", name="mb")
        nmb = cp.tile([128, 1], F32, tag="nmb", name="nmb")
        nm256 = cp.tile([128, 1], F32, tag="nm256", name="nm256")
        w_sb = cp.tile([64, 64], F32, tag="w_sb", name="w_sb")
        U_sb = cp.tile([1, OUT], F32, tag="U_sb", name="U_sb")
        m01 = cp.tile([1, 1], F32, tag="m01", name="m01")

        with tc.tile_pool(name="unpack", bufs=1) as up:
            # m chain first (tiny, unlocks q8 bias)
            usum = up.tile([1, NCORES * OUT], F32, tag="usum", name="usum")
            nc.sync.dma_start(out=usum[:, :],
                              in_=agc_out[0:NCORES, OFF_U:OFF_U + OUT])
            nc.vector.tensor_reduce(
                U_sb[:, :],
                usum[:, :].rearrange("a (k o) -> a o k", k=NCORES),
                axis=AX.X, op=OP.add)
            usq = up.tile([1, OUT], F32, tag="usq", name="usq")
            uu = up.tile([1, 1], F32, tag="uu", name="uu")
            nc.scalar.activation(usq[:, :], U_sb[:, :], AF.Square,
                                 accum_out=uu[:, :])
            nc.vector.tensor_scalar(m01[:, :], uu[:, :], -float(N), INV_N2,
                                    OP.add, OP.mult)
            nc.gpsimd.partition_broadcast(mb[:, :], m01[:, :])
            nc.vector.tensor_scalar(nmb[:, :], mb[:, :], -1.0, None, OP.mult)
            nc.vector.tensor_scalar(nm256[:, :], mb[:, :], -QS, None, OP.mult)
            # bulk unpack, all on the sync queue in consumption order
            for k in range(NCORES):
                nc.sync.dma_start(out=tf_blk[k][:, :].bitcast(F32),
                                  in_=agc_out[k:k + 1, OFF_T:OFF_T + 24576])
                nc.sync.dma_start(out=e8pack[k][:, :].bitcast(F32),
                                  in_=agc_out[k:k + 1, OFF_E:OFF_E + 24576])
            # w sum (needed only post-relation, for tw)
            wsum = up.tile([64, NCORES * 64], F32, tag="wsum", name="wsum")
            for k in range(NCORES):
                nc.sync.dma_start(
                    out=wsum[:, 64 * k:64 * (k + 1)],
                    in_=agc_out[k:k + 1, OFF_W:OFF_W + 4096])
            nc.vector.tensor_reduce(
                w_sb[:, :],
                wsum[:, :].rearrange("p (k o) -> p o k", k=NCORES),
                axis=AX.X, op=OP.add)

        # ---------- fused relation + propagation pass ----------
        qeT16 = cp.tile([128, P], F16, tag="qeT", name="qeT")
        ceT16 = cp.tile([128, P], F16, tag="ceT", name="ceT")
        with tc.tile_pool(name="ps_acc", bufs=1, space="PSUM") as pacc:
            qe_ps = [pacc.tile([128, 384], F32, tag=f"qe{h}", name=f"qe{h}")
                     for h in range(2)]
            ce_ps = [pacc.tile([128, 384], F32, tag=f"ce{h}", name=f"ce{h}")
                     for h in range(2)]
            with tc.tile_pool(name="ps_re", bufs=4, space="PSUM") as psre:
                for k in range(NCORES):
                    for pr in range(NT // 2):
                        q8p = [qcp.tile([128, 768], F8, tag=f"q8p{h}",
                                        name=f"q8p{h}") for h in range(2)]
                        c8p = [qcp.tile([128, 768], F8, tag=f"c8p{h}",
                                        name=f"c8p{h}") for h in range(2)]
                        for d in range(2):
                            sub = 2 * pr + d
                            j = NT * k + sub
                            stk = tf_blk[k][:, 128 * sub:128 * (sub + 1)]
                            for h in range(2):
                                sl = slice(384 * h, 384 * (h + 1))
                                dsl = slice(384 * d, 384 * (d + 1))
                                pp = psre.tile([128, 384], F32, tag="re",
                                               name="rem")
                                nc.tensor.matmul(pp[:, :], stk,
                                                 tT_stack[:, sl],
                                                 start=True, stop=True)
                                nc.scalar.activation(q8p[h][:, dsl], pp[:, :],
                                                     AF.Relu,
                                                     bias=nm256[:, :],
                                                     scale=QS)
                                slot = 2 * j + h
                                nc.vector.tensor_reduce(
                                    maxs[:, slot:slot + 1], pp[:, :],
                                    axis=AX.X, op=OP.max)
                        # c = [q8 > 0] from the scaled fp8 tiles
                        for h in range(2):
                            nc.vector.tensor_scalar(
                                c8p[h][:, :], q8p[h][:, :], 1e-4, None,
                                OP.is_ge)
                        first = (k == 0 and pr == 0)
                        last = (k == NCORES - 1 and pr == NT // 2 - 1)
                        lhs8 = e8pack[k][:, 256 * pr:256 * (pr + 1)].rearrange(
                            "a (two m) -> a two m", two=2)
                        for h in range(2):
                            rq = q8p[h][:, :].rearrange("a (two n) -> a two n",
                                                        two=2)
                            rc = c8p[h][:, :].rearrange("a (two n) -> a two n",
                                                        two=2)
                            nc.tensor.matmul(
                                qe_ps[h][:, :], lhs8, rq,
                                start=first, stop=last,
                                perf_mode=PM.DoubleRow,
                                skip_group_check=True)
                            nc.tensor.matmul(
                                ce_ps[h][:, :], lhs8, rc,
                                start=first, stop=last,
                                perf_mode=PM.DoubleRow,
                                skip_group_check=True)
            # qe/ce out of PSUM (fp16) so the psre pool can close
            for h in range(2):
                sl = slice(384 * h, 384 * (h + 1))
                nc.scalar.activation(qeT16[:, sl], qe_ps[h][:, :], AF.Copy,
                                     scale=1.0 / QS)
                nc.scalar.copy(ceT16[:, sl], ce_ps[h][:, :])

        # ---------- max stat -> AG2 ----------
        with tc.tile_pool(name="stats", bufs=1) as stp, \
             tc.tile_pool(name="ps_st", bufs=1, space="PSUM") as psst:
            ptm = psst.tile([108, 128], F32, tag="mtr", name="mtr")
            nc.tensor.transpose(ptm[:, :], maxs[:, :], ident32[:, :])
            mm_ = stp.tile([108, 128], F32, tag="mm", name="mm")
            nc.vector.tensor_scalar(mm_[:, :], ptm[:, :], colmask[:, :],
                                    None, OP.mult)
            mv = stp.tile([108, 1], F32, tag="mv", name="mv")
            nc.vector.tensor_reduce(mv[:, :], mm_[:, :], axis=AX.X,
                                    op=OP.max)
            ptm2 = psst.tile([1, 108], F32, tag="mtr2", name="mtr2")
            nc.tensor.transpose(ptm2[:, :], mv[:, :],
                                ident32[0:108, 0:108])
            mrow = stp.tile([1, 108], F32, tag="mrow", name="mrow")
            nc.scalar.copy(mrow[:, :], ptm2[:, :])
            mx01 = stp.tile([1, 1], F32, tag="mx01", name="mx01")
            nc.vector.tensor_reduce(mx01[:, :], mrow[:, :], axis=AX.X,
                                    op=OP.max)
            nc.sync.dma_start(out=ag2_in[0:1, 0:1], in_=mx01[:, :])
        nc.gpsimd.collective_compute(
            "AllGather", OP.bypass,
            ins=[ag2_in[:, :]],
            outs=[ag2_out[:, :]],
            replica_groups=[list(range(NCORES))])

        # ---------- AG2-window fill: tw, transposes, pre-ip epilogue ----
        if True:
            with tc.tile_pool(name="epi", bufs=1) as ep, \
                 tc.tile_pool(name="ps_epi", bufs=2, space="PSUM") as pse:
                # scalars derived from m
                imb = cp.tile([128, 1], F32, tag="imb", name="imb")
                nimb = cp.tile([128, 1], F32, tag="nimb", name="nimb")
                n1m = cp.tile([128, 1], F32, tag="n1m", name="n1m")
                nc.vector.reciprocal(imb[:, :], mb[:, :])
                nc.vector.tensor_scalar(nimb[:, :], imb[:, :], -1.0, None,
                                        OP.mult)
                nc.vector.tensor_scalar(n1m[:, :], mb[:, :], 1.0, -1.0,
                                        OP.mult, OP.add)
                # tw = t @ w  (fp16 t-hi)
                w16 = ep.tile([64, 64], F16, tag="w16", name="w16")
                nc.vector.tensor_copy(w16[:, :], w_sb[:, :])
                tw_nm = ep.tile([128, NT * OUT], F32, tag="tw", name="tw")
                for i in range(NT):
                    ptw = pse.tile([128, OUT], F32, tag="ptw", name="ptw")
                    nc.tensor.matmul(ptw[:, :],
                                     tT_stack[:, 128 * i:128 * (i + 1)],
                                     w16[:, :], start=True, stop=True)
                    nc.scalar.copy(tw_nm[:, esl(i)], ptw[:, :])
                # qe/ce to node-major via fp16 transposes; hi+lo plane add.
                # identS = I/256 folds the q8 unscale into the qe transpose.
                qe_nm = ep.tile([128, NT * OUT], F32, tag="qe_nm",
                                name="qe_nm")
                ce_nm = ep.tile([128, NT * OUT], F32, tag="ce_nm",
                                name="ce_nm")
                for i in range(NT):
                    csl = slice(128 * i, 128 * (i + 1))
                    pq = pse.tile([128, 128], F16, tag="tq", name="tq")
                    nc.tensor.transpose(pq[:, :], qeT16[:, csl],
                                        ident16[:, :])
                    qtmp = ep.tile([128, 128], F16, tag="qtmp", name="qtmp")
                    nc.scalar.copy(qtmp[:, :], pq[:, :])
                    nc.vector.tensor_add(qe_nm[:, esl(i)], qtmp[:, 0:64],
                                         qtmp[:, 64:128])
                    pc = pse.tile([128, 128], F16, tag="tc", name="tc")
                    nc.tensor.transpose(pc[:, :], ceT16[:, csl],
                                        ident16[:, :])
                    ctmp = ep.tile([128, 128], F16, tag="ctmp", name="ctmp")
                    nc.scalar.copy(ctmp[:, :], pc[:, :])
                    nc.vector.tensor_add(ce_nm[:, esl(i)], ctmp[:, 0:64],
                                         ctmp[:, 64:128])
                # G = (qe - tw)/m + ce ;  z1 = qe + (m-1)*emb
                G = ep.tile([128, NT * OUT], F32, tag="G", name="G")
                nc.vector.scalar_tensor_tensor(G[:, :], qe_nm[:, :],
                                               imb[:, :], ce_nm[:, :],
                                               op0=OP.mult, op1=OP.add)
                nc.vector.scalar_tensor_tensor(G[:, :], tw_nm[:, :],
                                               nimb[:, :], G[:, :],
                                               op0=OP.mult, op1=OP.add)
                z1 = ep.tile([128, NT * OUT], F32, tag="z1", name="z1")
                nc.vector.scalar_tensor_tensor(z1[:, :], emb_loc[:, :],
                                               n1m[:, :], qe_nm[:, :],
                                               op0=OP.mult, op1=OP.add)
                cpos = ep.tile([128, NT * OUT], F32, tag="cpos", name="cpos")
                nc.vector.scalar_tensor_tensor(cpos[:, :], G[:, :], aH[:, :],
                                               emb_loc[:, :], op0=OP.mult,
                                               op1=OP.add)
                cneg = ep.tile([128, NT * OUT], F32, tag="cneg", name="cneg")
                nc.vector.scalar_tensor_tensor(cneg[:, :], emb_loc[:, :],
                                               naH[:, :], G[:, :],
                                               op0=OP.mult, op1=OP.subtract)
                emb_half = ep.tile([128, NT * OUT], F32, tag="ehalf",
                                   name="ehalf")
                nc.vector.tensor_scalar(emb_half[:, :], emb_loc[:, :], 0.5,
                                        None, OP.mult)
                # pre-warm the Exp activation table during the AG2 window
                dmx = ep.tile([1, 1], F32, tag="dmx", name="dmx")
                nc.scalar.activation(dmx[:, :], m01[:, :], AF.Exp)

                # ---------- AG2 -> ip ----------
                ipb = cp.tile([128, 1], F32, tag="ipb", name="ipb")
                naip = cp.tile([128, 1], F32, tag="naip", name="naip")
                with tc.tile_pool(name="glob", bufs=1) as gp:
                    m8 = gp.tile([1, NCORES], F32, tag="m8", name="m8")
                    nc.sync.dma_start(out=m8[:, :], in_=ag2_out[:, 0:1])
                    mxg = gp.tile([1, 1], F32, tag="mxg", name="mxg")
                    nc.vector.tensor_reduce(mxg[:, :], m8[:, :], axis=AX.X,
                                            op=OP.max)
                    pd = gp.tile([1, 1], F32, tag="pd", name="pd")
                    nc.vector.tensor_sub(pd[:, :], mxg[:, :], m01[:, :])
                    ip01 = gp.tile([1, 1], F32, tag="ip01", name="ip01")
                    nc.vector.reciprocal(ip01[:, :], pd[:, :])
                    nc.gpsimd.partition_broadcast(ipb[:, :], ip01[:, :])
                    nc.vector.tensor_mul(naip[:, :], ipb[:, :], naH[:, :])

                pos_w = ep.tile([128, NT * OUT], F32, tag="pos_w",
                                name="pos_w")
                nc.vector.scalar_tensor_tensor(pos_w[:, :], z1[:, :],
                                               ipb[:, :], cpos[:, :],
                                               op0=OP.mult, op1=OP.add)
                neg_w = ep.tile([128, NT * OUT], F32, tag="neg_w",
                                name="neg_w")
                nc.vector.scalar_tensor_tensor(neg_w[:, :], z1[:, :],
                                               naip[:, :], cneg[:, :],
                                               op0=OP.mult, op1=OP.add)

                pp_w = ep.tile([128, NT * OUT], F32, tag="pp_w", name="pp_w")
                pn_w = ep.tile([128, NT * OUT], F32, tag="pn_w", name="pn_w")
                for src_w, dst in ((pos_w, pp_w), (neg_w, pn_w)):
                    rmx2 = ep.tile([128, NT], F32, tag="rmx2", name="rmx2")
                    nc.vector.tensor_reduce(
                        rmx2[:, :],
                        src_w[:, :].rearrange("p (g o) -> p g o", o=OUT),
                        axis=AX.X, op=OP.max, negate=True)
                    ex2 = ep.tile([128, NT * OUT], F32, tag="ex2", name="ex2")
                    ssum2 = ep.tile([128, NT], F32, tag="ssum2", name="ssum2")
                    for i in range(NT):
                        nc.scalar.activation(ex2[:, esl(i)], src_w[:, esl(i)],
                                             AF.Exp, bias=rmx2[:, i:i + 1],
                                             scale=1.0)
                    nc.vector.tensor_reduce(
                        ssum2[:, :],
                        ex2[:, :].rearrange("p (g o) -> p g o", o=OUT),
                        axis=AX.X, op=OP.add)
                    rs2 = ep.tile([128, NT], F32, tag="rs2", name="rs2")
                    nc.vector.reciprocal(rs2[:, :], ssum2[:, :])
                    for i in range(NT):
                        nc.vector.tensor_scalar(dst[:, esl(i)], ex2[:, esl(i)],
                                                rs2[:, i:i + 1], None, OP.mult)
                dd = ep.tile([128, NT * OUT], F32, tag="dd", name="dd")
                nc.vector.scalar_tensor_tensor(dd[:, :], pp_w[:, :], 0.5,
                                               emb_half[:, :], op0=OP.mult,
                                               op1=OP.add)
                msg = ep.tile([128, NT * OUT], F32, tag="msg", name="msg")
                nc.vector.scalar_tensor_tensor(msg[:, :], pn_w[:, :], -0.5,
                                               dd[:, :], op0=OP.mult,
                                               op1=OP.add)
                for i in range(NT):
                    eng = (nc.gpsimd, nc.sync, nc.scalar)[i % 3]
                    eng.dma_start(out=out_msg[128 * i:128 * (i + 1), :],
                                  in_=msg[:, esl(i)])

    nc.compile()
    return nc


def _get_nc():
    if "nc" not in _CACHE:
        _CACHE["nc"] = _build()
    return _CACHE["nc"]


def _make_in_maps(inputs):
    f = np.float32
    h = np.float16
    sm = np.ascontiguousarray(inputs["smoothed_feature"], dtype=f)
    ori = np.ascontiguousarray(inputs["ori_feature"], dtype=f)
    aM = float(np.asarray(inputs["prelu_model"]).reshape(-1)[0])
    aHv = float(np.asarray(inputs["prelu_hete"]).reshape(-1)[0])

    def b0pair(name):
        b = np.ascontiguousarray(inputs[name], dtype=f)
        return b, np.ascontiguousarray(-b, dtype=f)

    bh0v, nbh0v = b0pair("b_hete0")
    bs0v, nbs0v = b0pair("b_smooth0")
    bl0v, nbl0v = b0pair("b_local0")

    def b1b(name):
        return np.ascontiguousarray(
            np.broadcast_to(np.asarray(inputs[name], dtype=f), (128, OUT)))

    shared = {
        "Wh0": inputs["W_hete0"].astype(h), "Wh1": inputs["W_hete1"].astype(h),
        "Ws0": inputs["W_smooth0"].astype(h),
        "Ws1": inputs["W_smooth1"].astype(h),
        "Wl0": inputs["W_local0"].astype(h),
        "Wl1": inputs["W_local1"].astype(h),
        "bh0": bh0v, "nbh0": nbh0v, "bs0": bs0v, "nbs0": nbs0v,
        "bl0": bl0v, "nbl0": nbl0v,
        "bh1b": b1b("b_hete1"), "bs1b": b1b("b_smooth1"),
        "bl1b": b1b("b_local1"),
        "naH_b": np.full((128, 1), -aHv, dtype=f),
        "naM_b": np.full((128, 1), -aM, dtype=f),
        "aH_b": np.full((128, 1), aHv, dtype=f),
        "ident16": np.eye(128, dtype=h),
        "identS": (np.eye(128) / QS).astype(h),
        "ident32": np.eye(128, dtype=f),
        "ones_col": np.ones((128, 1), dtype=f),
    }
    shared = {k: np.ascontiguousarray(v) for k, v in shared.items()}
    in_maps = []
    for r in range(NCORES):
        cm = np.ones(108, dtype=f)
        cm[2 * NT * r:2 * NT * (r + 1)] = 0.0  # drop raw own-block slots
        m = dict(shared)
        m["xT_sm"] = np.ascontiguousarray(
            sm[P * r:P * (r + 1)].T.astype(h))
        m["xT_ori"] = np.ascontiguousarray(
            ori[P * r:P * (r + 1)].T.astype(h))
        m["colmask"] = cm
        in_maps.append(m)
    return in_maps


def _ensure_ntff_hook():
    """The agent image's antenv lacks axon_hooks; shim it so
    run_bass_kernel_spmd(trace=True) can capture NTFF profiles."""
    if "antenv.axon_hooks" in sys.modules:
        return
    import types
    import antenv
    mod = types.ModuleType("antenv.axon_hooks")
    state = {"hook": None}
    mod.set_axon_ntff_profile_hook = lambda h: state.__setitem__("hook", h)
    mod.get_axon_ntff_profile_hook = lambda: state["hook"]
    sys.modules["antenv.axon_hooks"] = mod
    antenv.axon_hooks = mod
    try:
        from trn_agent_boot.trn_boot import _ntff_profile_via_ctypes
        mod.set_axon_ntff_profile_hook(
            _ntff_profile_via_ctypes("/opt/axon/libaxon_pjrt.so"))
    except Exception as e:
        print(f"ntff hook install failed: {e}", file=sys.stderr)


def run(inputs, trace=False):
    if trace:
        _ensure_ntff_hook()
    nc = _get_nc()
    in_maps = _make_in_maps(inputs)
    res = run_bass_kernel_spmd(nc, in_maps, list(range(NCORES)), trace=trace)
    outs = res.results
    o1 = np.concatenate([outs[r]["out_ori"] for r in range(NCORES)], axis=0)
    o2 = np.concatenate([outs[r]["out_smooth"] for r in range(NCORES)], axis=0)
    o3 = np.concatenate([outs[r]["out_msg"] for r in range(NCORES)], axis=0)
    return (o1.astype(np.float32), o2.astype(np.float32),
            o3.astype(np.float32)), res


def kernel(**inputs):
    (o1, o2, o3), _ = run(inputs, trace=False)
    return (o1, o2, o3)
